# revision 3
# baseline (speedup 1.0000x reference)
"""Trainium2 Bass kernel for DeepGraphGO-style 2-layer GraphConv model.

  x1 = relu(features @ W1 + b1)
  x2 = GraphConv(x1; src1, dst1, Wc1, bc1)   # D_in^-1/2 A D_out^-1/2 x W + b
  x3 = GraphConv(x2; src2, dst2, Wc2, bc2)
  out = sigmoid(x3 @ W2 + b2)

Sharding: nodes are padded to 20480 and split contiguously across 8 cores
(2560 nodes per core, 20 blocks of 128).  Each core computes its node shard
through every layer; the per-layer "message" tensors g = (x @ Wc) * deg_out^-1/2
are AllGathered so every core can gather arbitrary source rows.  The
segment-sum is computed per 128-node destination block as a sequence of
one-hot selection matmuls on the tensor engine (edges are host-sorted by
destination).  All normalization factors are exact f32 per-partition scales;
matmul operands are bf16 with f32 PSUM accumulation.
"""

import math
import os
from dataclasses import dataclass

import numpy as np
import ml_dtypes

import concourse.bass as bass
import concourse.bacc as bacc
import concourse.tile as tile
from concourse import mybir
from concourse.masks import make_identity
from concourse.bass_utils import run_bass_kernel_spmd

BF16 = ml_dtypes.bfloat16
P = 128


@dataclass(frozen=True)
class Cfg:
    n_nodes: int = 20000          # real nodes
    n_cores: int = 8
    nb: int = 20                  # 128-node blocks per core
    fin: int = 2048               # input feature dim
    h: int = 1024                 # hidden dim
    go: int = 5000                # output dim

    @property
    def npc(self):                # nodes per core (padded)
        return self.nb * P

    @property
    def n_pad(self):
        return self.n_cores * self.npc

    @property
    def ki(self):                 # fin 128-chunks
        return self.fin // P

    @property
    def kh(self):                 # h 128-chunks
        return self.h // P


FULL = Cfg()


# ---------------------------------------------------------------- host prep

def _tile_kmaj(w, k_chunks, ncols):
    """[k_chunks*128, ncols] -> [128, k_chunks*ncols] with dev[p, k*ncols+j] = w[k*128+p, j]."""
    return np.ascontiguousarray(
        w.reshape(k_chunks, P, ncols).transpose(1, 0, 2).reshape(P, k_chunks * ncols)
    )


def _edge_prep(cfg, src, dst, cpb=None):
    """Per-core edge structures for one conv layer.

    Returns (cpb, per_core list of (idx_dev int16 [128, nb*cpb*8],
    wsel_dev bf16 [128, nb*cpb*128])).
    """
    npc, nb = cfg.npc, cfg.nb
    per_core_edges = []
    max_cnt = 0
    for c in range(cfg.n_cores):
        sel = (dst >= c * npc) & (dst < (c + 1) * npc)
        s_e = src[sel].astype(np.int64)
        d_e = (dst[sel] - c * npc).astype(np.int64)
        order = np.argsort(d_e, kind="stable")
        s_e, d_e = s_e[order], d_e[order]
        blk = d_e // P
        counts = np.bincount(blk, minlength=nb)
        max_cnt = max(max_cnt, int(counts.max()))
        per_core_edges.append((s_e, d_e, blk, counts))
    need_cpb = math.ceil(max_cnt / P)
    if cpb is None:
        cpb = need_cpb
    assert cpb >= need_cpb
    npad = cpb * P

    out = []
    for s_e, d_e, blk, counts in per_core_edges:
        starts = np.zeros(nb + 1, np.int64)
        np.cumsum(counts, out=starts[1:])
        idx_flat = np.zeros((nb, npad), np.int64)        # gather row ids (0 pad)
        wsel = np.zeros((nb, npad, P), np.float32)       # one-hot per edge
        for b in range(nb):
            cnt = int(counts[b])
            if cnt == 0:
                continue
            sl = slice(starts[b], starts[b + 1])
            idx_flat[b, :cnt] = s_e[sl]
            wsel[b, np.arange(cnt), d_e[sl] - b * P] = 1.0
        # device wsel layout: [128(edge k), nb*cpb*128] ; dev[k, b, j, m] = wsel[b, j*128+k, m]
        wsel_dev = np.ascontiguousarray(
            wsel.reshape(nb, cpb, P, P).transpose(2, 0, 1, 3).reshape(P, nb * cpb * P)
        ).astype(BF16)
        # idx layout: wrapped into 16 partitions, replicated x8
        x = idx_flat.reshape(nb, cpb * 8, 16).transpose(2, 0, 1).reshape(16, nb * cpb * 8)
        idx_dev = np.ascontiguousarray(np.tile(x, (8, 1))).astype(np.int16)
        out.append((idx_dev, wsel_dev))
    return cpb, out


def prep_inputs(cfg, inputs):
    """Build the SPMD per-core input maps. Returns (cpb, in_maps)."""
    f32 = np.float32
    feats = np.asarray(inputs["features"], f32)
    W1 = np.asarray(inputs["W1"], f32)
    Wc1 = np.asarray(inputs["Wc1"], f32)
    Wc2 = np.asarray(inputs["Wc2"], f32)
    W2 = np.asarray(inputs["W2"], f32)
    for bname in ("b1", "bc1", "bc2", "b2"):
        assert not np.any(np.asarray(inputs[bname])), f"nonzero bias {bname} unsupported"
    src1 = np.asarray(inputs["src1"]).astype(np.int64)
    dst1 = np.asarray(inputs["dst1"]).astype(np.int64)
    src2 = np.asarray(inputs["src2"]).astype(np.int64)
    dst2 = np.asarray(inputs["dst2"]).astype(np.int64)

    npc, nb, n_pad = cfg.npc, cfg.nb, cfg.n_pad

    deg_out1 = np.maximum(np.bincount(src1, minlength=n_pad), 1.0).astype(f32) ** -0.5
    deg_in1 = np.maximum(np.bincount(dst1, minlength=n_pad), 1.0).astype(f32) ** -0.5
    deg_out2 = np.maximum(np.bincount(src2, minlength=n_pad), 1.0).astype(f32) ** -0.5
    deg_in2 = np.maximum(np.bincount(dst2, minlength=n_pad), 1.0).astype(f32) ** -0.5

    featp = np.zeros((n_pad, cfg.fin), f32)
    featp[: cfg.n_nodes] = feats

    w1_dev = _tile_kmaj(W1, cfg.ki, cfg.h).astype(BF16)
    wc1_dev = _tile_kmaj(Wc1, cfg.kh, cfg.h).astype(BF16)
    wc2_dev = _tile_kmaj(Wc2, cfg.kh, cfg.h).astype(BF16)
    w2_dev = _tile_kmaj(W2, cfg.kh, cfg.go).astype(BF16)

    cpb1, e1 = _edge_prep(cfg, src1, dst1)
    cpb2, e2 = _edge_prep(cfg, src2, dst2)
    cpb = max(cpb1, cpb2)
    if cpb1 < cpb:
        _, e1 = _edge_prep(cfg, src1, dst1, cpb)
    if cpb2 < cpb:
        _, e2 = _edge_prep(cfg, src2, dst2, cpb)

    in_maps = []
    for c in range(cfg.n_cores):
        lo, hi = c * npc, (c + 1) * npc
        featT = featp[lo:hi].T  # [fin, npc]
        featT_dev = _tile_kmaj(np.ascontiguousarray(featT), cfg.ki, npc).astype(BF16)
        s1 = deg_out1[lo:hi].reshape(nb, P).T                      # g1 row scale
        s2 = (deg_in1[lo:hi] * deg_out2[lo:hi]).reshape(nb, P).T   # g2 row scale
        s3 = deg_in2[lo:hi].reshape(nb, P).T                       # final scale
        s_all = np.ascontiguousarray(np.concatenate([s1, s2, s3], axis=1)).astype(f32)
        in_maps.append(
            {
                "featT": featT_dev,
                "w1": w1_dev,
                "wc1": wc1_dev,
                "wc2": wc2_dev,
                "w2": w2_dev,
                "s_all": s_all,
                "idx1": e1[c][0],
                "wsel1": e1[c][1],
                "idx2": e2[c][0],
                "wsel2": e2[c][1],
            }
        )
    return cpb, in_maps


# ---------------------------------------------------------------- device build

def build_bass(cfg, cpb, phases=4):
    f32, bf16, i16 = mybir.dt.float32, mybir.dt.bfloat16, mybir.dt.int16
    nb, npc, ki, kh, h, go = cfg.nb, cfg.npc, cfg.ki, cfg.kh, cfg.h, cfg.go
    ngrp = npc // 512

    nc = bacc.Bacc("TRN2", target_bir_lowering=False, debug=False, num_devices=cfg.n_cores)

    featT = nc.dram_tensor("featT", [P, ki * npc], bf16, kind="ExternalInput")
    w1 = nc.dram_tensor("w1", [P, ki * h], bf16, kind="ExternalInput")
    wc1 = nc.dram_tensor("wc1", [P, kh * h], bf16, kind="ExternalInput")
    wc2 = nc.dram_tensor("wc2", [P, kh * h], bf16, kind="ExternalInput")
    w2 = nc.dram_tensor("w2", [P, kh * go], bf16, kind="ExternalInput")
    s_all = nc.dram_tensor("s_all", [P, 3 * nb], f32, kind="ExternalInput")
    idx1 = nc.dram_tensor("idx1", [P, nb * cpb * 8], i16, kind="ExternalInput")
    wsel1 = nc.dram_tensor("wsel1", [P, nb * cpb * P], bf16, kind="ExternalInput")
    idx2 = nc.dram_tensor("idx2", [P, nb * cpb * 8], i16, kind="ExternalInput")
    wsel2 = nc.dram_tensor("wsel2", [P, nb * cpb * P], bf16, kind="ExternalInput")
    out_d = nc.dram_tensor("out", [npc, go], f32, kind="ExternalOutput")

    ag1_in = nc.dram_tensor("ag1_in", [npc, h], bf16, kind="Internal")
    ag1_out = nc.dram_tensor("ag1_out", [cfg.n_pad, h], bf16, kind="Internal", addr_space="Shared")
    ag2_in = nc.dram_tensor("ag2_in", [npc, h], bf16, kind="Internal")
    ag2_out = nc.dram_tensor("ag2_out", [cfg.n_pad, h], bf16, kind="Internal", addr_space="Shared")

    rg = [list(range(cfg.n_cores))]
    mult = mybir.AluOpType.mult
    Relu = mybir.ActivationFunctionType.Relu
    Sigmoid = mybir.ActivationFunctionType.Sigmoid

    # final-phase output column groups (multiples of 512 except the last)
    fgroups = []
    gstart = 0
    while gstart < go:
        gn = min(2048, go - gstart)
        fgroups.append((gstart, gn))
        gstart += gn

    def conv_layer(tc, ident, ag_out_t, idx_sb, wsel_t, dstT_sb):
        """Scatter conv: dstT_sb[:, m, b*128+n] = sum_{e: dst=b*128+n} g_full[src_e, m*128+p].

        The one-hot W_sel is the stationary operand (one weight load per edge
        chunk, N=512 streaming); the node-major PSUM aggregate is then
        transposed back to feature-major via PE transpose-mode matmuls.
        """
        nc_ = tc.nc
        with tc.tile_pool(name="gat", bufs=2) as gat_p, \
             tc.tile_pool(name="wsl", bufs=4) as wsl_p, \
             tc.tile_pool(name="agg", bufs=2) as agg_p, \
             tc.tile_pool(name="cps", bufs=2, space="PSUM") as cps_p, \
             tc.tile_pool(name="tps", bufs=2, space="PSUM") as tps_p:
            for b in range(nb):
                gt = gat_p.tile([P, cpb, h], mybir.dt.bfloat16, tag="gt")
                for j0 in range(0, cpb, 6):
                    jn = min(6, cpb - j0)
                    nc_.gpsimd.dma_gather(
                        gt[:, j0:j0 + jn, :], ag_out_t[:],
                        idx_sb[:, (b * cpb + j0) * 8:(b * cpb + j0 + jn) * 8],
                        jn * P, jn * P, h,
                    )
                ws = wsl_p.tile([P, cpb * P], mybir.dt.bfloat16, tag="ws")
                nc_.sync.dma_start(out=ws[:], in_=wsel_t[:, b * cpb * P:(b + 1) * cpb * P])
                ps = cps_p.tile([P, h], mybir.dt.float32, tag="cps")
                for j in range(cpb):
                    for hh in range(h // 512):
                        nc_.tensor.matmul(
                            ps[:, hh * 512:(hh + 1) * 512],
                            lhsT=ws[:, j * P:(j + 1) * P],
                            rhs=gt[:, j, hh * 512:(hh + 1) * 512],
                            start=(j == 0),
                            stop=(j == cpb - 1),
                        )
                agg = agg_p.tile([P, h], mybir.dt.bfloat16, tag="agg")
                nc_.vector.tensor_copy(out=agg[:], in_=ps[:])
                for m in range(kh):
                    tp = tps_p.tile([P, P], mybir.dt.bfloat16, tag="tps")
                    nc_.tensor.transpose(
                        out=tp[:], in_=agg[:, m * P:(m + 1) * P], identity=ident[:]
                    )
                    nc_.vector.tensor_copy(out=dstT_sb[:, m, b * P:(b + 1) * P], in_=tp[:])

    def gemm_nodeblocks(tc, lhsT_sb, w_sb, s_col, ag_in_t, pool_ps, pool_sb):
        """g[b] = (x @ Wc) * s  per 128-node block; DMA to ag_in_t rows."""
        nc_ = tc.nc
        for b in range(nb):
            ps2 = pool_ps.tile([P, h], mybir.dt.float32, tag="gps")
            for k in range(kh):
                for hh in range(h // 512):
                    nc_.tensor.matmul(
                        ps2[:, hh * 512:(hh + 1) * 512],
                        lhsT=lhsT_sb[:, k, b * P:(b + 1) * P],
                        rhs=w_sb[:, k, hh * 512:(hh + 1) * 512],
                        start=(k == 0),
                        stop=(k == kh - 1),
                    )
            gsb = pool_sb.tile([P, h], mybir.dt.bfloat16, tag="gsb")
            nc_.vector.tensor_scalar(
                out=gsb[:], in0=ps2[:], scalar1=s_col(b), scalar2=None, op0=mult
            )
            nc_.sync.dma_start(out=ag_in_t[b * P:(b + 1) * P, :], in_=gsb[:])

    with tile.TileContext(nc) as tc:
        with tc.tile_pool(name="consts", bufs=1) as consts:
            s_sb = consts.tile([P, 3 * nb], f32)
            nc.sync.dma_start(out=s_sb[:], in_=s_all[:])
            idx1_sb = consts.tile([P, nb * cpb * 8], i16)
            nc.sync.dma_start(out=idx1_sb[:], in_=idx1[:])
            idx2_sb = consts.tile([P, nb * cpb * 8], i16)
            nc.sync.dma_start(out=idx2_sb[:], in_=idx2[:])
            ident = consts.tile([P, P], bf16)
            make_identity(nc, ident[:])

            # ---------------- phase 1: x1T = relu(W1^T featT); g1 = (x1 @ Wc1) * s1
            with tc.tile_pool(name="ph1", bufs=1) as ph1, \
                 tc.tile_pool(name="ft", bufs=2) as ft_p, \
                 tc.tile_pool(name="ps1", bufs=4, space="PSUM") as ps1_p, \
                 tc.tile_pool(name="gout", bufs=2) as gout_p:
                w1_sb = ph1.tile([P, ki, h], bf16)
                nc.sync.dma_start(out=w1_sb[:], in_=w1[:].rearrange("p (k n) -> p k n", k=ki))
                wc1_sb = ph1.tile([P, kh, h], bf16)
                nc.sync.dma_start(out=wc1_sb[:], in_=wc1[:].rearrange("p (k n) -> p k n", k=kh))
                h1T_sb = ph1.tile([P, kh, npc], bf16)
                featT_r = featT[:].rearrange("p (k n) -> p k n", k=ki)
                for g in range(ngrp):
                    ft = ft_p.tile([P, ki, 512], bf16, tag="ft")
                    nc.sync.dma_start(out=ft[:], in_=featT_r[:, :, g * 512:(g + 1) * 512])
                    for m in range(kh):
                        ps = ps1_p.tile([P, 512], f32, tag="ps1")
                        for k in range(ki):
                            nc.tensor.matmul(
                                ps[:],
                                lhsT=w1_sb[:, k, m * P:(m + 1) * P],
                                rhs=ft[:, k, :],
                                start=(k == 0),
                                stop=(k == ki - 1),
                            )
                        nc.scalar.activation(
                            out=h1T_sb[:, m, g * 512:(g + 1) * 512], in_=ps[:], func=Relu
                        )
                with tc.tile_pool(name="gps1", bufs=2, space="PSUM") as gps_p:
                    gemm_nodeblocks(
                        tc, h1T_sb, wc1_sb, lambda b: s_sb[:, b:b + 1], ag1_in, gps_p, gout_p
                    )

            nc.gpsimd.collective_compute(
                "AllGather", mybir.AluOpType.bypass,
                ins=[ag1_in[:]], outs=[ag1_out[:]], replica_groups=rg,
            )

            # ---------------- phase 2: conv1 -> x2T ; g2 = (x2 @ Wc2) * s2
            if phases >= 2:
                with tc.tile_pool(name="ph2", bufs=1) as ph2, \
                     tc.tile_pool(name="gout2", bufs=2) as gout2_p:
                    wc2_sb = ph2.tile([P, kh, h], bf16)
                    nc.sync.dma_start(out=wc2_sb[:], in_=wc2[:].rearrange("p (k n) -> p k n", k=kh))
                    x2T_sb = ph2.tile([P, kh, npc], bf16)
                    conv_layer(tc, ident, ag1_out, idx1_sb, wsel1, x2T_sb)
                    with tc.tile_pool(name="gps2", bufs=2, space="PSUM") as gps2_p:
                        gemm_nodeblocks(
                            tc, x2T_sb, wc2_sb, lambda b: s_sb[:, nb + b:nb + b + 1],
                            ag2_in, gps2_p, gout2_p,
                        )

                nc.gpsimd.collective_compute(
                    "AllGather", mybir.AluOpType.bypass,
                    ins=[ag2_in[:]], outs=[ag2_out[:]], replica_groups=rg,
                )

            # ---------------- phase 3: conv2 -> x3T
            if phases >= 3:
                with tc.tile_pool(name="ph3", bufs=1) as ph3:
                    x3T_sb = ph3.tile([P, kh, npc], bf16)
                    conv_layer(tc, ident, ag2_out, idx2_sb, wsel2, x3T_sb)

                    # ------------- phase 4: out = sigmoid(s3 * (x3 @ W2))
                    if phases >= 4:
                        w2_r = w2[:].rearrange("p (k n) -> p k n", k=kh)
                        with tc.tile_pool(name="w2p", bufs=2) as w2_p, \
                             tc.tile_pool(name="fps", bufs=2, space="PSUM") as fps_p, \
                             tc.tile_pool(name="fout", bufs=3) as fout_p:
                            for gstart, gn in fgroups:
                                w2g = w2_p.tile([P, kh, 2048], bf16, tag="w2g")
                                nc.sync.dma_start(out=w2g[:, :, :gn], in_=w2_r[:, :, gstart:gstart + gn])
                                for b in range(nb):
                                    ps = fps_p.tile([P, 2048], f32, tag="fps")
                                    for k in range(kh):
                                        for cs in range(0, gn, 512):
                                            cn = min(512, gn - cs)
                                            nc.tensor.matmul(
                                                ps[:, cs:cs + cn],
                                                lhsT=x3T_sb[:, k, b * P:(b + 1) * P],
                                                rhs=w2g[:, k, cs:cs + cn],
                                                start=(k == 0),
                                                stop=(k == kh - 1),
                                            )
                                    o = fout_p.tile([P, 2048], f32, tag="fo")
                                    nc.scalar.activation(
                                        out=o[:, :gn], in_=ps[:, :gn], func=Sigmoid,
                                        scale=s_sb[:, 2 * nb + b:2 * nb + b + 1],
                                    )
                                    nc.sync.dma_start(
                                        out=out_d[b * P:(b + 1) * P, gstart:gstart + gn],
                                        in_=o[:, :gn],
                                    )

    nc.compile()
    return nc


# ---------------------------------------------------------------- entry point

def _ensure_ntff_hook():
    """Register the axon NTFF profile hook if the image's antenv lacks it."""
    import contextlib
    import ctypes
    import sys
    import types

    try:
        from antenv.axon_hooks import get_axon_ntff_profile_hook  # noqa: F401
        return
    except ImportError:
        pass
    try:
        import antenv
    except ImportError:
        return
    mod = types.ModuleType("antenv.axon_hooks")
    holder = [None]
    mod.set_axon_ntff_profile_hook = lambda h: holder.__setitem__(0, h)
    mod.get_axon_ntff_profile_hook = lambda: holder[0]
    sys.modules["antenv.axon_hooks"] = mod
    antenv.axon_hooks = mod
    try:
        lib = ctypes.CDLL("/opt/axon/libaxon_pjrt.so")
    except OSError:
        return
    if not hasattr(lib, "axon_start_nrt_profile"):
        return
    lib.axon_start_nrt_profile.argtypes = [
        ctypes.POINTER(ctypes.c_int64),
        ctypes.c_size_t,
    ]
    lib.axon_start_nrt_profile.restype = ctypes.c_int64
    lib.axon_stop_nrt_profile.argtypes = [ctypes.c_char_p]
    lib.axon_stop_nrt_profile.restype = ctypes.c_int64

    @contextlib.contextmanager
    def _hook(output_dir, device_ids):
        import jax

        jax.devices()
        if device_ids:
            ids = (ctypes.c_int64 * len(device_ids))(*device_ids)
            rc = lib.axon_start_nrt_profile(ids, len(device_ids))
        else:
            rc = lib.axon_start_nrt_profile(None, 0)
        if rc != 0:
            raise RuntimeError(f"axon_start_nrt_profile rc={rc}")
        try:
            yield
        finally:
            n = lib.axon_stop_nrt_profile(str(output_dir).encode())
            print(f"profile: {n} file(s) written to {output_dir}", file=sys.stderr)

    holder[0] = _hook


def _run_hw(cfg, inputs, trace=False):
    if trace:
        _ensure_ntff_hook()
    cpb, in_maps = prep_inputs(cfg, inputs)
    phases = int(os.environ.get("GNN_PHASES", "4"))
    nc = build_bass(cfg, cpb, phases=phases)
    res = run_bass_kernel_spmd(nc, in_maps, core_ids=list(range(cfg.n_cores)), trace=trace)
    full = np.concatenate([res.results[c]["out"] for c in range(cfg.n_cores)], axis=0)
    return full[: cfg.n_nodes], res


def kernel(**inputs) -> np.ndarray:
    trace = bool(int(os.environ.get("GNN_TRACE", "0")))
    out, res = _run_hw(FULL, inputs, trace=trace)
    if trace and res.exec_time_ns is not None:
        print(f"HW exec time: {res.exec_time_ns} ns")
    return out



# revision 18
# speedup vs baseline: 1.1361x; 1.1361x over previous
"""Trainium2 Bass kernel for DeepGraphGO-style 2-layer GraphConv model.

  x1 = relu(features @ W1 + b1)
  x2 = GraphConv(x1; src1, dst1, Wc1, bc1)   # D_in^-1/2 A D_out^-1/2 x W + b
  x3 = GraphConv(x2; src2, dst2, Wc2, bc2)
  out = sigmoid(x3 @ W2 + b2)

Sharding: nodes are padded to 20480 and split contiguously across 8 cores
(2560 nodes per core, 20 blocks of 128).  Each core computes its node shard
through every layer; the per-layer "message" tensors g = (x @ Wc) * deg_out^-1/2
are quantized to fp8-e4m3 and AllGathered so every core can gather arbitrary
source rows (1 KB/row).  The segment-sum is computed per 128-node destination
block as a sequence of one-hot selection matmuls on the tensor engine (edges
are host-sorted by destination); the one-hot matrices are built on-device from
compact per-edge destination-column ids (iota + is_equal on the vector
engine).  All normalization factors are exact f32 per-partition scales; dense
matmul operands are bf16 with f32 PSUM accumulation; the final output is
written bf16 and upcast on host.

The whole kernel is a per-128-node-block pipeline: gather(b) -> one-hot
scatter matmuls(b) -> PE transpose(b) -> next-layer GEMM(b) -> DMA, with the
final x3 @ W2 GEMM interleaved per block into the conv2 loop so tensor-engine
work overlaps gather DMA.
"""

import math
import os
from dataclasses import dataclass

import numpy as np
import ml_dtypes

import concourse.bass as bass
import concourse.bacc as bacc
import concourse.tile as tile
from concourse import mybir
from concourse.masks import make_identity
from concourse.bass_utils import run_bass_kernel_spmd

BF16 = ml_dtypes.bfloat16
FP8 = ml_dtypes.float8_e4m3
P = 128


@dataclass(frozen=True)
class Cfg:
    n_nodes: int = 20000          # real nodes
    n_cores: int = 8
    nb: int = 20                  # 128-node blocks per core
    fin: int = 2048               # input feature dim
    h: int = 1024                 # hidden dim
    go: int = 5000                # output dim

    @property
    def npc(self):                # nodes per core (padded)
        return self.nb * P

    @property
    def n_pad(self):
        return self.n_cores * self.npc

    @property
    def ki(self):                 # fin 128-chunks
        return self.fin // P

    @property
    def kh(self):                 # h 128-chunks
        return self.h // P


FULL = Cfg()


# ---------------------------------------------------------------- host prep

def _tile_kmaj(w, k_chunks, ncols):
    """[k_chunks*128, ncols] -> [128, k_chunks*ncols] with dev[p, k*ncols+j] = w[k*128+p, j]."""
    return np.ascontiguousarray(
        w.reshape(k_chunks, P, ncols).transpose(1, 0, 2).reshape(P, k_chunks * ncols)
    )


def _edge_prep(cfg, src, dst, cpb=None):
    """Per-core edge structures for one conv layer.

    Returns (cpb, per_core list of (idx_dev int16 [128, nb*cpb*8],
    dcol_dev f32 [128, nb*cpb])).
    """
    npc, nb = cfg.npc, cfg.nb
    per_core_edges = []
    max_cnt = 0
    for c in range(cfg.n_cores):
        sel = (dst >= c * npc) & (dst < (c + 1) * npc)
        s_e = src[sel].astype(np.int64)
        d_e = (dst[sel] - c * npc).astype(np.int64)
        order = np.argsort(d_e, kind="stable")
        s_e, d_e = s_e[order], d_e[order]
        blk = d_e // P
        counts = np.bincount(blk, minlength=nb)
        max_cnt = max(max_cnt, int(counts.max()))
        per_core_edges.append((s_e, d_e, blk, counts))
    need_cpb = math.ceil(max_cnt / P)
    if cpb is None:
        cpb = need_cpb
    assert cpb >= need_cpb
    npad = cpb * P

    out = []
    for s_e, d_e, blk, counts in per_core_edges:
        starts = np.zeros(nb + 1, np.int64)
        np.cumsum(counts, out=starts[1:])
        idx_flat = np.zeros((nb, npad), np.int64)        # gather row ids (0 pad)
        dcol = np.full((nb, npad), -1.0, np.float32)     # within-block dst col (-1 pad)
        for b in range(nb):
            cnt = int(counts[b])
            if cnt == 0:
                continue
            sl = slice(starts[b], starts[b + 1])
            idx_flat[b, :cnt] = s_e[sl]
            dcol[b, :cnt] = (d_e[sl] - b * P).astype(np.float32)
        # dcol device layout: [128(edge lane), nb*cpb]; dev[p, b*cpb+j] = dcol[b, j*128+p]
        dcol_dev = np.ascontiguousarray(
            dcol.reshape(nb, cpb, P).transpose(2, 0, 1).reshape(P, nb * cpb)
        )
        # idx layout: wrapped into 16 partitions, replicated x8
        x = idx_flat.reshape(nb, cpb * 8, 16).transpose(2, 0, 1).reshape(16, nb * cpb * 8)
        idx_dev = np.ascontiguousarray(np.tile(x, (8, 1))).astype(np.int16)
        out.append((idx_dev, dcol_dev))
    return cpb, out


def prep_inputs(cfg, inputs):
    """Build the SPMD per-core input maps. Returns (cpb, in_maps)."""
    f32 = np.float32
    feats = np.asarray(inputs["features"], f32)
    W1 = np.asarray(inputs["W1"], f32)
    Wc1 = np.asarray(inputs["Wc1"], f32)
    Wc2 = np.asarray(inputs["Wc2"], f32)
    W2 = np.asarray(inputs["W2"], f32)
    for bname in ("b1", "bc1", "bc2", "b2"):
        assert not np.any(np.asarray(inputs[bname])), f"nonzero bias {bname} unsupported"
    src1 = np.asarray(inputs["src1"]).astype(np.int64)
    dst1 = np.asarray(inputs["dst1"]).astype(np.int64)
    src2 = np.asarray(inputs["src2"]).astype(np.int64)
    dst2 = np.asarray(inputs["dst2"]).astype(np.int64)

    npc, nb, n_pad = cfg.npc, cfg.nb, cfg.n_pad

    deg_out1 = np.maximum(np.bincount(src1, minlength=n_pad), 1.0).astype(f32) ** -0.5
    deg_in1 = np.maximum(np.bincount(dst1, minlength=n_pad), 1.0).astype(f32) ** -0.5
    deg_out2 = np.maximum(np.bincount(src2, minlength=n_pad), 1.0).astype(f32) ** -0.5
    deg_in2 = np.maximum(np.bincount(dst2, minlength=n_pad), 1.0).astype(f32) ** -0.5

    featp = np.zeros((n_pad, cfg.fin), f32)
    featp[: cfg.n_nodes] = feats

    w1_dev = _tile_kmaj(W1, cfg.ki, cfg.h).astype(BF16)
    wc1_dev = _tile_kmaj(Wc1, cfg.kh, cfg.h).astype(BF16)
    wc2_dev = _tile_kmaj(Wc2, cfg.kh, cfg.h).astype(BF16)
    w2_dev = _tile_kmaj(W2, cfg.kh, cfg.go).astype(BF16)

    cpb1, e1 = _edge_prep(cfg, src1, dst1)
    cpb2, e2 = _edge_prep(cfg, src2, dst2)
    cpb = max(cpb1, cpb2)
    cpb += cpb % 2  # even chunk count for DoubleRow pairing
    if cpb1 < cpb:
        _, e1 = _edge_prep(cfg, src1, dst1, cpb)
    if cpb2 < cpb:
        _, e2 = _edge_prep(cfg, src2, dst2, cpb)

    in_maps = []
    for c in range(cfg.n_cores):
        lo, hi = c * npc, (c + 1) * npc
        featT = featp[lo:hi].T  # [fin, npc]
        featT_dev = _tile_kmaj(np.ascontiguousarray(featT), cfg.ki, npc).astype(BF16)
        s1 = deg_out1[lo:hi].reshape(nb, P).T                      # g1 row scale
        s2 = (deg_in1[lo:hi] * deg_out2[lo:hi]).reshape(nb, P).T   # g2 row scale
        s3 = deg_in2[lo:hi].reshape(nb, P).T                       # final scale
        s_all = np.ascontiguousarray(np.concatenate([s1, s2, s3], axis=1)).astype(f32)
        in_maps.append(
            {
                "featT": featT_dev,
                "w1": w1_dev,
                "wc1": wc1_dev,
                "wc2": wc2_dev,
                "w2": w2_dev,
                "s_all": s_all,
                "idx1": e1[c][0],
                "dcol1": e1[c][1],
                "idx2": e2[c][0],
                "dcol2": e2[c][1],
            }
        )
    return cpb, in_maps


# ---------------------------------------------------------------- device build

def build_bass(cfg, cpb, phases=4):
    f32, bf16, i16 = mybir.dt.float32, mybir.dt.bfloat16, mybir.dt.int16
    f8, i32 = mybir.dt.float8e4, mybir.dt.int32
    nb, npc, ki, kh, h, go = cfg.nb, cfg.npc, cfg.ki, cfg.kh, cfg.h, cfg.go
    ngrp = npc // 512

    n_swq = int(os.environ.get("GNN_SWQ", "1"))
    nc = bacc.Bacc(
        "TRN2", target_bir_lowering=False, debug=False,
        num_devices=cfg.n_cores, num_swdge_queues=n_swq,
    )

    featT = nc.dram_tensor("featT", [P, ki * npc], bf16, kind="ExternalInput")
    w1 = nc.dram_tensor("w1", [P, ki * h], bf16, kind="ExternalInput")
    wc1 = nc.dram_tensor("wc1", [P, kh * h], bf16, kind="ExternalInput")
    wc2 = nc.dram_tensor("wc2", [P, kh * h], bf16, kind="ExternalInput")
    w2 = nc.dram_tensor("w2", [P, kh * go], bf16, kind="ExternalInput")
    s_all = nc.dram_tensor("s_all", [P, 3 * nb], f32, kind="ExternalInput")
    idx1 = nc.dram_tensor("idx1", [P, nb * cpb * 8], i16, kind="ExternalInput")
    dcol1 = nc.dram_tensor("dcol1", [P, nb * cpb], f32, kind="ExternalInput")
    idx2 = nc.dram_tensor("idx2", [P, nb * cpb * 8], i16, kind="ExternalInput")
    dcol2 = nc.dram_tensor("dcol2", [P, nb * cpb], f32, kind="ExternalInput")
    out_d = nc.dram_tensor("out", [npc, go], bf16, kind="ExternalOutput")

    ag1_in = nc.dram_tensor("ag1_in", [npc, h], f8, kind="Internal")
    ag1_out = nc.dram_tensor("ag1_out", [cfg.n_pad, h], f8, kind="Internal", addr_space="Shared")
    ag2_in = nc.dram_tensor("ag2_in", [npc, h], f8, kind="Internal")
    ag2_out = nc.dram_tensor("ag2_out", [cfg.n_pad, h], f8, kind="Internal", addr_space="Shared")

    rg = [list(range(cfg.n_cores))]
    mult = mybir.AluOpType.mult
    is_eq = mybir.AluOpType.is_equal
    Relu = mybir.ActivationFunctionType.Relu
    Sigmoid = mybir.ActivationFunctionType.Sigmoid

    # final-phase output column groups
    fgroups = []
    gstart = 0
    while gstart < go:
        gn = min(2048, go - gstart)
        fgroups.append((gstart, gn))
        gstart += gn

    use_dr = bool(int(os.environ.get("GNN_DR", "1")))

    def build_ws(ws, iota_w, dcol_sb, b):
        """One-hot scatter matrices for dst block b: ws[p, j, m] = (dcol[p, b*cpb+j] == m)."""
        nc.vector.tensor_tensor(
            out=ws[:], in0=iota_w[:],
            in1=dcol_sb[:, b * cpb:(b + 1) * cpb].broadcast_to([P, cpb, P]),
            op=is_eq,
        )

    def gather_block(gt, ag_out_t, idx_sb, b, _sems):
        """SWDGE gather of block b's padded edge rows (fp8 via bf16 view)."""
        for j0 in range(0, cpb, 6):
            jn = min(6, cpb - j0)
            nc.gpsimd.dma_gather(
                gt[:, j0:j0 + jn, :].bitcast(bf16),
                ag_out_t[:].bitcast(bf16),
                idx_sb[:, (b * cpb + j0) * 8:(b * cpb + j0 + jn) * 8],
                jn * P, jn * P, h // 2,
            )

    def conv_block(gt, ws, ident, xb, cps_p, tps_p, agg_p):
        """One dst block: scatter matmuls + transpose back to feature-major xb."""
        ps = cps_p.tile([P, h], f32, tag="cps")
        if use_dr:
            for j2 in range(0, cpb, 2):
                for hh in range(h // 512):
                    nc.tensor.matmul(
                        ps[:, hh * 512:(hh + 1) * 512],
                        lhsT=ws[:, j2:j2 + 2, :],
                        rhs=gt[:, j2:j2 + 2, hh * 512:(hh + 1) * 512],
                        start=(j2 == 0),
                        stop=(j2 == cpb - 2),
                        perf_mode=mybir.MatmulPerfMode.DoubleRow,
                    )
        else:
            for j in range(cpb):
                for hh in range(h // 512):
                    nc.tensor.matmul(
                        ps[:, hh * 512:(hh + 1) * 512],
                        lhsT=ws[:, j, :],
                        rhs=gt[:, j, hh * 512:(hh + 1) * 512],
                        start=(j == 0),
                        stop=(j == cpb - 1),
                    )
        agg = agg_p.tile([P, h], bf16, tag="agg")
        nc.vector.tensor_copy(out=agg[:], in_=ps[:])
        for m in range(kh):
            tp = tps_p.tile([P, P], bf16, tag="tps")
            nc.tensor.transpose(out=tp[:], in_=agg[:, m * P:(m + 1) * P], identity=ident[:])
            nc.vector.tensor_copy(out=xb[:, m, :], in_=tp[:])

    with tile.TileContext(nc) as tc:
        with tc.tile_pool(name="consts", bufs=1) as consts:
            s_sb = consts.tile([P, 3 * nb], f32)
            nc.sync.dma_start(out=s_sb[:], in_=s_all[:])
            idx1_sb = consts.tile([P, nb * cpb * 8], i16)
            nc.sync.dma_start(out=idx1_sb[:], in_=idx1[:])
            idx2_sb = consts.tile([P, nb * cpb * 8], i16)
            nc.sync.dma_start(out=idx2_sb[:], in_=idx2[:])
            dcol1_sb = consts.tile([P, nb * cpb], f32)
            nc.sync.dma_start(out=dcol1_sb[:], in_=dcol1[:])
            dcol2_sb = consts.tile([P, nb * cpb], f32)
            nc.sync.dma_start(out=dcol2_sb[:], in_=dcol2[:])
            ident = consts.tile([P, P], bf16)
            make_identity(nc, ident[:])
            # iota_w[p, j, m] = m  (f32; values 0..127 are exact)
            iota_w = consts.tile([P, cpb, P], f32)
            nc.gpsimd.iota(
                iota_w[:], pattern=[[0, cpb], [1, P]], base=0,
                channel_multiplier=0, allow_small_or_imprecise_dtypes=True,
            )
            gat_sems = [nc.alloc_semaphore(f"gat_dma{i}") for i in range(16)]

            # ------------- phase 1: x1 = relu(W1^T featT) by 512-col groups;
            # g1[b] = (x1[b] @ Wc1) * s1[b] interleaved per 4-block group
            with tc.tile_pool(name="ph1", bufs=1) as ph1, \
                 tc.tile_pool(name="ft", bufs=2) as ft_p, \
                 tc.tile_pool(name="h1g", bufs=2) as h1g_p, \
                 tc.tile_pool(name="ps1", bufs=4, space="PSUM") as ps1_p, \
                 tc.tile_pool(name="gps1", bufs=1, space="PSUM") as gps1_p, \
                 tc.tile_pool(name="gout", bufs=2) as gout_p:
                w1_sb = ph1.tile([P, ki, h], bf16)
                nc.sync.dma_start(out=w1_sb[:], in_=w1[:].rearrange("p (k n) -> p k n", k=ki))
                wc1_sb = ph1.tile([P, kh, h], bf16)
                nc.sync.dma_start(out=wc1_sb[:], in_=wc1[:].rearrange("p (k n) -> p k n", k=kh))
                featT_r = featT[:].rearrange("p (k n) -> p k n", k=ki)
                for g in range(ngrp):
                    ft = ft_p.tile([P, ki, 512], bf16, tag="ft")
                    nc.sync.dma_start(out=ft[:], in_=featT_r[:, :, g * 512:(g + 1) * 512])
                    h1g = h1g_p.tile([P, kh, 512], bf16, tag="h1g")
                    for m in range(kh):
                        ps = ps1_p.tile([P, 512], f32, tag="ps1")
                        for k in range(ki):
                            nc.tensor.matmul(
                                ps[:],
                                lhsT=w1_sb[:, k, m * P:(m + 1) * P],
                                rhs=ft[:, k, :],
                                start=(k == 0),
                                stop=(k == ki - 1),
                            )
                        nc.scalar.activation(out=h1g[:, m, :], in_=ps[:], func=Relu)
                    for bq in range(4):
                        b = g * 4 + bq
                        ps2 = gps1_p.tile([P, h], f32, tag="gps")
                        for k in range(kh):
                            for hh in range(h // 512):
                                nc.tensor.matmul(
                                    ps2[:, hh * 512:(hh + 1) * 512],
                                    lhsT=h1g[:, k, bq * P:(bq + 1) * P],
                                    rhs=wc1_sb[:, k, hh * 512:(hh + 1) * 512],
                                    start=(k == 0),
                                    stop=(k == kh - 1),
                                )
                        gsb = gout_p.tile([P, h], f8, tag="gsb")
                        nc.vector.tensor_scalar(
                            out=gsb[:], in0=ps2[:], scalar1=s_sb[:, b:b + 1],
                            scalar2=None, op0=mult,
                        )
                        nc.sync.dma_start(out=ag1_in[b * P:(b + 1) * P, :], in_=gsb[:])

            nc.gpsimd.collective_compute(
                "AllGather", mybir.AluOpType.bypass,
                ins=[ag1_in[:]], outs=[ag1_out[:]], replica_groups=rg,
            )

            # ------------- phases 2-4 share the resident W2 tile
            if phases >= 2:
                with tc.tile_pool(name="ph234", bufs=1) as ph234:
                    # W2 resident for phases 2-4; loaded here so the DMA is
                    # done long before AG2 needs the wires
                    w2_sb = ph234.tile([P, kh, go], bf16)
                    w2_r = w2[:].rearrange("p (k n) -> p k n", k=kh)
                    for gstart, gn in fgroups:
                        nc.sync.dma_start(
                            out=w2_sb[:, :, gstart:gstart + gn],
                            in_=w2_r[:, :, gstart:gstart + gn],
                        )

                    # ----- phase 2: conv1 per block -> x2[b]; g2[b] = (x2[b] @ Wc2) * s2[b]
                    with tc.tile_pool(name="ph2", bufs=1) as ph2, \
                         tc.tile_pool(name="gat", bufs=2) as gat_p, \
                         tc.tile_pool(name="wsl", bufs=2) as wsl_p, \
                         tc.tile_pool(name="agg", bufs=2) as agg_p, \
                         tc.tile_pool(name="x2b", bufs=3) as x2b_p, \
                         tc.tile_pool(name="gout2", bufs=2) as gout2_p, \
                         tc.tile_pool(name="cps", bufs=2, space="PSUM") as cps_p, \
                         tc.tile_pool(name="tps", bufs=2, space="PSUM") as tps_p, \
                         tc.tile_pool(name="gps2", bufs=1, space="PSUM") as gps2_p:
                        wc2_sb = ph2.tile([P, kh, h], bf16)
                        nc.sync.dma_start(out=wc2_sb[:], in_=wc2[:].rearrange("p (k n) -> p k n", k=kh))
                        for b in range(nb):
                            gt = gat_p.tile([P, cpb, h], f8, tag="gt")
                            gather_block(gt, ag1_out, idx1_sb, b, gat_sems)
                            ws = wsl_p.tile([P, cpb, P], f8, tag="ws")
                            build_ws(ws, iota_w, dcol1_sb, b)
                            x2b = x2b_p.tile([P, kh, P], bf16, tag="x2b")
                            conv_block(gt, ws, ident, x2b, cps_p, tps_p, agg_p)
                            ps2 = gps2_p.tile([P, h], f32, tag="g2ps")
                            for k in range(kh):
                                for hh in range(h // 512):
                                    nc.tensor.matmul(
                                        ps2[:, hh * 512:(hh + 1) * 512],
                                        lhsT=x2b[:, k, :],
                                        rhs=wc2_sb[:, k, hh * 512:(hh + 1) * 512],
                                        start=(k == 0),
                                        stop=(k == kh - 1),
                                    )
                            gsb = gout2_p.tile([P, h], f8, tag="gsb2")
                            nc.vector.tensor_scalar(
                                out=gsb[:], in0=ps2[:], scalar1=s_sb[:, nb + b:nb + b + 1],
                                scalar2=None, op0=mult,
                            )
                            nc.sync.dma_start(out=ag2_in[b * P:(b + 1) * P, :], in_=gsb[:])

                    nc.gpsimd.collective_compute(
                        "AllGather", mybir.AluOpType.bypass,
                        ins=[ag2_in[:]], outs=[ag2_out[:]], replica_groups=rg,
                    )

                    # ----- phase 3+4: conv2 per block -> x3[b]; out[b] = sigmoid(s3*(x3[b] @ W2))
                    if phases >= 3:
                        with tc.tile_pool(name="gat3", bufs=2) as gat3_p, \
                             tc.tile_pool(name="wsl3", bufs=2) as wsl3_p, \
                             tc.tile_pool(name="agg3", bufs=2) as agg3_p, \
                             tc.tile_pool(name="x3b", bufs=3) as x3b_p, \
                             tc.tile_pool(name="fout", bufs=3) as fout_p, \
                             tc.tile_pool(name="cps3", bufs=2, space="PSUM") as cps3_p, \
                             tc.tile_pool(name="tps3", bufs=2, space="PSUM") as tps3_p, \
                             tc.tile_pool(name="fps", bufs=2, space="PSUM") as fps_p:
                            for b in range(nb):
                                gt = gat3_p.tile([P, cpb, h], f8, tag="gt3")
                                gather_block(gt, ag2_out, idx2_sb, b, gat_sems)
                                ws = wsl3_p.tile([P, cpb, P], f8, tag="ws3")
                                build_ws(ws, iota_w, dcol2_sb, b)
                                x3b = x3b_p.tile([P, kh, P], bf16, tag="x3b")
                                conv_block(gt, ws, ident, x3b, cps3_p, tps3_p, agg3_p)
                                if phases >= 4:
                                    for gstart, gn in fgroups:
                                        o = fout_p.tile([P, 2048], bf16, tag="fo")
                                        for cs in range(0, gn, 512):
                                            cn = min(512, gn - cs)
                                            ps4 = fps_p.tile([P, 512], f32, tag="fps")
                                            for k in range(kh):
                                                nc.tensor.matmul(
                                                    ps4[:, :cn],
                                                    lhsT=x3b[:, k, :],
                                                    rhs=w2_sb[:, k, gstart + cs:gstart + cs + cn],
                                                    start=(k == 0),
                                                    stop=(k == kh - 1),
                                                )
                                            nc.scalar.activation(
                                                out=o[:, cs:cs + cn], in_=ps4[:, :cn], func=Sigmoid,
                                                scale=s_sb[:, 2 * nb + b:2 * nb + b + 1],
                                            )
                                        nc.sync.dma_start(
                                            out=out_d[b * P:(b + 1) * P, gstart:gstart + gn],
                                            in_=o[:, :gn],
                                        )

    nc.compile()
    return nc


# ---------------------------------------------------------------- entry point

def _ensure_ntff_hook():
    """Register the axon NTFF profile hook if the image's antenv lacks it."""
    import contextlib
    import ctypes
    import sys
    import types

    try:
        from antenv.axon_hooks import get_axon_ntff_profile_hook  # noqa: F401
        return
    except ImportError:
        pass
    try:
        import antenv
    except ImportError:
        return
    mod = types.ModuleType("antenv.axon_hooks")
    holder = [None]
    mod.set_axon_ntff_profile_hook = lambda h: holder.__setitem__(0, h)
    mod.get_axon_ntff_profile_hook = lambda: holder[0]
    sys.modules["antenv.axon_hooks"] = mod
    antenv.axon_hooks = mod
    try:
        lib = ctypes.CDLL("/opt/axon/libaxon_pjrt.so")
    except OSError:
        return
    if not hasattr(lib, "axon_start_nrt_profile"):
        return
    lib.axon_start_nrt_profile.argtypes = [
        ctypes.POINTER(ctypes.c_int64),
        ctypes.c_size_t,
    ]
    lib.axon_start_nrt_profile.restype = ctypes.c_int64
    lib.axon_stop_nrt_profile.argtypes = [ctypes.c_char_p]
    lib.axon_stop_nrt_profile.restype = ctypes.c_int64

    @contextlib.contextmanager
    def _hook(output_dir, device_ids):
        import jax

        jax.devices()
        if device_ids:
            ids = (ctypes.c_int64 * len(device_ids))(*device_ids)
            rc = lib.axon_start_nrt_profile(ids, len(device_ids))
        else:
            rc = lib.axon_start_nrt_profile(None, 0)
        if rc != 0:
            raise RuntimeError(f"axon_start_nrt_profile rc={rc}")
        try:
            yield
        finally:
            n = lib.axon_stop_nrt_profile(str(output_dir).encode())
            print(f"profile: {n} file(s) written to {output_dir}", file=sys.stderr)

    holder[0] = _hook


def _run_hw(cfg, inputs, trace=False):
    if trace:
        _ensure_ntff_hook()
    cpb, in_maps = prep_inputs(cfg, inputs)
    phases = int(os.environ.get("GNN_PHASES", "4"))
    nc = build_bass(cfg, cpb, phases=phases)
    res = run_bass_kernel_spmd(nc, in_maps, core_ids=list(range(cfg.n_cores)), trace=trace)
    full = np.concatenate(
        [np.asarray(res.results[c]["out"]).astype(np.float32) for c in range(cfg.n_cores)],
        axis=0,
    )
    return full[: cfg.n_nodes], res


def kernel(**inputs) -> np.ndarray:
    trace = bool(int(os.environ.get("GNN_TRACE", "0")))
    out, res = _run_hw(FULL, inputs, trace=trace)
    if trace and res.exec_time_ns is not None:
        print(f"HW exec time: {res.exec_time_ns} ns")
    return out


# revision 26
# speedup vs baseline: 1.2100x; 1.0650x over previous
"""Trainium2 Bass kernel for DeepGraphGO-style 2-layer GraphConv model.

  x1 = relu(features @ W1 + b1)
  x2 = GraphConv(x1; src1, dst1, Wc1, bc1)   # D_in^-1/2 A D_out^-1/2 x W + b
  x3 = GraphConv(x2; src2, dst2, Wc2, bc2)
  out = sigmoid(x3 @ W2 + b2)

Sharding: nodes are padded to 20480 and split contiguously across 8 cores
(2560 nodes per core, 20 blocks of 128).  Each core computes its node shard
through every layer; the per-layer "message" tensors g = (x @ Wc) * deg_out^-1/2
are quantized to fp8-e4m3 and AllGathered so every core can gather arbitrary
source rows (1 KB/row).

The per-edge gather is bound by SWDGE descriptor generation on the GpSimd Q7
(~10 ns/row, engine-serial), so the kernel is organized to keep that engine
busy continuously: each layer's AllGather is split into two node-half
collectives (first half fires as soon as the first 10 blocks' messages are
ready, overlapping the producing phase), and each destination block's edges
are host-sorted by (dst, src-half) so gathers for the first half start before
the second collective lands.

The segment-sum is computed per 128-node destination block as one-hot
selection matmuls on the tensor engine in fp8 DoubleRow mode (256 edges per
pass); one-hot matrices are built on-device (iota + is_equal on the vector
engine) from compact per-edge destination-column ids.  The final x3 @ W2 GEMM
is interleaved per block into the conv2 loop so tensor-engine work overlaps
gather DMA; output is written bf16 and upcast on host.
"""

import math
import os
from dataclasses import dataclass

import numpy as np
import ml_dtypes

import concourse.bass as bass
import concourse.bacc as bacc
import concourse.tile as tile
from concourse import mybir
from concourse.masks import make_identity
from concourse.bass_utils import run_bass_kernel_spmd

BF16 = ml_dtypes.bfloat16
FP8 = ml_dtypes.float8_e4m3
P = 128


@dataclass(frozen=True)
class Cfg:
    n_nodes: int = 20000          # real nodes
    n_cores: int = 8
    nb: int = 20                  # 128-node blocks per core
    fin: int = 2048               # input feature dim
    h: int = 1024                 # hidden dim
    go: int = 5000                # output dim

    @property
    def npc(self):                # nodes per core (padded)
        return self.nb * P

    @property
    def nh(self):                 # nodes per core half
        return self.npc // 2

    @property
    def n_pad(self):
        return self.n_cores * self.npc

    @property
    def ki(self):                 # fin 128-chunks
        return self.fin // P

    @property
    def kh(self):                 # h 128-chunks
        return self.h // P


FULL = Cfg()


# ---------------------------------------------------------------- host prep

def _tile_kmaj(w, k_chunks, ncols):
    """[k_chunks*128, ncols] -> [128, k_chunks*ncols] with dev[p, k*ncols+j] = w[k*128+p, j]."""
    return np.ascontiguousarray(
        w.reshape(k_chunks, P, ncols).transpose(1, 0, 2).reshape(P, k_chunks * ncols)
    )


def _edge_prep(cfg, src, dst, cpb=None):
    """Per-core edge structures for one conv layer, with per-destination-block
    edges grouped by source node-half (half A: src%npc < nh).

    Returns (cpb, meta, per_core list of (idx_dev int16 [128, nb*cpb*8],
    dcol_dev f32 [128, nb*cpb])).  meta = (maxA, maxB, ncA, ncB): per-block
    max-over-cores real edge counts per half and chunk counts (shared across
    cores).  Gather row ids index the half buffer: core*nh + (src%npc) - half*nh.
    """
    npc, nb, nh = cfg.npc, cfg.nb, cfg.nh
    per_core = []
    for c in range(cfg.n_cores):
        sel = (dst >= c * npc) & (dst < (c + 1) * npc)
        s_e = src[sel].astype(np.int64)
        d_e = (dst[sel] - c * npc).astype(np.int64)
        half = ((s_e % npc) >= nh).astype(np.int64)
        order = np.lexsort((half, d_e))
        s_e, d_e, half = s_e[order], d_e[order], half[order]
        blk = d_e // P
        cntA = np.bincount(blk[half == 0], minlength=nb)
        cntB = np.bincount(blk[half == 1], minlength=nb)
        per_core.append((s_e, d_e, half, cntA, cntB))

    maxA = [max(int(pc[3][b]) for pc in per_core) for b in range(nb)]
    maxB = [max(int(pc[4][b]) for pc in per_core) for b in range(nb)]
    ncA = [math.ceil(m / P) for m in maxA]
    ncB = [math.ceil(m / P) for m in maxB]
    # per-block gathered chunk count, padded even for DoubleRow pairing; the
    # pad chunks gather row 0 of the B half so every chunk the matmul reads
    # holds finite data (stale NaN x zero one-hot would still poison PSUM)
    ncE = [a + b + (a + b) % 2 for a, b in zip(ncA, ncB)]
    need_cpb = max(ncE)
    if cpb is None:
        cpb = need_cpb
    assert cpb >= need_cpb
    npad = cpb * P

    out = []
    for s_e, d_e, half, cntA, cntB in per_core:
        idx_flat = np.full((nb, npad), -1, np.int64)     # -1: skipped by ucode
        dcol = np.full((nb, npad), -1.0, np.float32)     # -1 pad -> all-zero ws row
        for b in range(nb):
            mb_ = (d_e // P) == b
            for hsel, nchunks, base in ((0, ncA[b], 0), (1, ncE[b] - ncA[b], ncA[b] * P)):
                m = mb_ & (half == hsel)
                cnt = int(m.sum())
                s_h = s_e[m]
                rows = (s_h // npc) * nh + (s_h % npc) - hsel * nh
                idx_flat[b, base:base + cnt] = rows
                idx_flat[b, base + cnt:base + nchunks * P] = 0   # pad: row 0 (finite)
                dcol[b, base:base + cnt] = (d_e[m] - b * P).astype(np.float32)
        # dcol device layout: [128(edge lane), nb*cpb]; dev[p, b*cpb+j] = dcol[b, j*128+p]
        dcol_dev = np.ascontiguousarray(
            dcol.reshape(nb, cpb, P).transpose(2, 0, 1).reshape(P, nb * cpb)
        )
        # idx layout: wrapped into 16 partitions, replicated x8
        x = idx_flat.reshape(nb, cpb * 8, 16).transpose(2, 0, 1).reshape(16, nb * cpb * 8)
        idx_dev = np.ascontiguousarray(np.tile(x, (8, 1))).astype(np.int16)
        out.append((idx_dev, dcol_dev))
    return cpb, (ncA, ncE), out


def prep_inputs(cfg, inputs):
    """Build the SPMD per-core input maps. Returns (cpb, metas, in_maps)."""
    f32 = np.float32
    feats = np.asarray(inputs["features"], f32)
    W1 = np.asarray(inputs["W1"], f32)
    Wc1 = np.asarray(inputs["Wc1"], f32)
    Wc2 = np.asarray(inputs["Wc2"], f32)
    W2 = np.asarray(inputs["W2"], f32)
    for bname in ("b1", "bc1", "bc2", "b2"):
        assert not np.any(np.asarray(inputs[bname])), f"nonzero bias {bname} unsupported"
    src1 = np.asarray(inputs["src1"]).astype(np.int64)
    dst1 = np.asarray(inputs["dst1"]).astype(np.int64)
    src2 = np.asarray(inputs["src2"]).astype(np.int64)
    dst2 = np.asarray(inputs["dst2"]).astype(np.int64)

    npc, nb, n_pad = cfg.npc, cfg.nb, cfg.n_pad

    deg_out1 = np.maximum(np.bincount(src1, minlength=n_pad), 1.0).astype(f32) ** -0.5
    deg_in1 = np.maximum(np.bincount(dst1, minlength=n_pad), 1.0).astype(f32) ** -0.5
    deg_out2 = np.maximum(np.bincount(src2, minlength=n_pad), 1.0).astype(f32) ** -0.5
    deg_in2 = np.maximum(np.bincount(dst2, minlength=n_pad), 1.0).astype(f32) ** -0.5

    featp = np.zeros((n_pad, cfg.fin), f32)
    featp[: cfg.n_nodes] = feats

    w1_dev = _tile_kmaj(W1, cfg.ki, cfg.h).astype(BF16)
    wc1_dev = _tile_kmaj(Wc1, cfg.kh, cfg.h).astype(BF16)
    wc2_dev = _tile_kmaj(Wc2, cfg.kh, cfg.h).astype(BF16)
    w2_dev = _tile_kmaj(W2, cfg.kh, cfg.go).astype(BF16)

    cpb1, m1, e1 = _edge_prep(cfg, src1, dst1)
    cpb2, m2, e2 = _edge_prep(cfg, src2, dst2)
    cpb = max(cpb1, cpb2)
    if cpb1 < cpb:
        _, m1, e1 = _edge_prep(cfg, src1, dst1, cpb)
    if cpb2 < cpb:
        _, m2, e2 = _edge_prep(cfg, src2, dst2, cpb)

    in_maps = []
    for c in range(cfg.n_cores):
        lo, hi = c * npc, (c + 1) * npc
        featT = featp[lo:hi].T  # [fin, npc]
        featT_dev = _tile_kmaj(np.ascontiguousarray(featT), cfg.ki, npc).astype(BF16)
        s1 = deg_out1[lo:hi].reshape(nb, P).T                      # g1 row scale
        s2 = (deg_in1[lo:hi] * deg_out2[lo:hi]).reshape(nb, P).T   # g2 row scale
        s3 = deg_in2[lo:hi].reshape(nb, P).T                       # final scale
        s_all = np.ascontiguousarray(np.concatenate([s1, s2, s3], axis=1)).astype(f32)
        in_maps.append(
            {
                "featT": featT_dev,
                "w1": w1_dev,
                "wc1": wc1_dev,
                "wc2": wc2_dev,
                "w2": w2_dev,
                "s_all": s_all,
                "idx1": e1[c][0],
                "dcol1": e1[c][1],
                "idx2": e2[c][0],
                "dcol2": e2[c][1],
            }
        )
    return cpb, (m1, m2), in_maps


# ---------------------------------------------------------------- device build

def build_bass(cfg, cpb, metas, phases=4):
    f32, bf16, i16 = mybir.dt.float32, mybir.dt.bfloat16, mybir.dt.int16
    f8 = mybir.dt.float8e4
    nb, npc, nh, ki, kh, h, go = cfg.nb, cfg.npc, cfg.nh, cfg.ki, cfg.kh, cfg.h, cfg.go
    ngrp = npc // 512
    nhb = nb // 2

    nc = bacc.Bacc("TRN2", target_bir_lowering=False, debug=False, num_devices=cfg.n_cores)

    featT = nc.dram_tensor("featT", [P, ki * npc], bf16, kind="ExternalInput")
    w1 = nc.dram_tensor("w1", [P, ki * h], bf16, kind="ExternalInput")
    wc1 = nc.dram_tensor("wc1", [P, kh * h], bf16, kind="ExternalInput")
    wc2 = nc.dram_tensor("wc2", [P, kh * h], bf16, kind="ExternalInput")
    w2 = nc.dram_tensor("w2", [P, kh * go], bf16, kind="ExternalInput")
    s_all = nc.dram_tensor("s_all", [P, 3 * nb], f32, kind="ExternalInput")
    idx1 = nc.dram_tensor("idx1", [P, nb * cpb * 8], i16, kind="ExternalInput")
    dcol1 = nc.dram_tensor("dcol1", [P, nb * cpb], f32, kind="ExternalInput")
    idx2 = nc.dram_tensor("idx2", [P, nb * cpb * 8], i16, kind="ExternalInput")
    dcol2 = nc.dram_tensor("dcol2", [P, nb * cpb], f32, kind="ExternalInput")
    out_d = nc.dram_tensor("out", [npc, go], bf16, kind="ExternalOutput")

    ag_in = {}
    ag_out = {}
    for layer in (1, 2):
        for hf in ("a", "b"):
            ag_in[layer, hf] = nc.dram_tensor(f"ag{layer}{hf}_in", [nh, h], f8, kind="Internal")
            ag_out[layer, hf] = nc.dram_tensor(
                f"ag{layer}{hf}_out", [cfg.n_cores * nh, h], f8,
                kind="Internal", addr_space="Shared",
            )

    mult = mybir.AluOpType.mult
    is_eq = mybir.AluOpType.is_equal
    Relu = mybir.ActivationFunctionType.Relu
    Sigmoid = mybir.ActivationFunctionType.Sigmoid
    DR = mybir.MatmulPerfMode.DoubleRow
    rg = [list(range(cfg.n_cores))]

    # final-phase output column groups
    fgroups = []
    gstart = 0
    while gstart < go:
        gn = min(2048, go - gstart)
        fgroups.append((gstart, gn))
        gstart += gn

    def build_ws(ws, iota_w, dcol_sb, b):
        """One-hot scatter matrices for dst block b: ws[p, j, m] = (dcol[p, b*cpb+j] == m)."""
        nc.vector.tensor_tensor(
            out=ws[:], in0=iota_w[:],
            in1=dcol_sb[:, b * cpb:(b + 1) * cpb].broadcast_to([P, cpb, P]),
            op=is_eq,
        )

    def gather_half(gt, ag_out_t, idx_sb, b, base, nch):
        """Gather one source-half of block b: chunks [base, base+nch) of gt."""
        for j0 in range(0, nch, 6):
            jn = min(6, nch - j0)
            nc.gpsimd.dma_gather(
                gt[:, base + j0:base + j0 + jn, :].bitcast(bf16),
                ag_out_t[:].bitcast(bf16),
                idx_sb[:, (b * cpb + base + j0) * 8:(b * cpb + base + j0 + jn) * 8],
                jn * P, jn * P, h // 2,
            )

    def conv_block(gt, ws, ident, xb, nc_b, cps_p, tps_p, agg_p):
        """One dst block: DoubleRow scatter matmuls + transpose to feature-major xb."""
        npair = nc_b // 2
        ps = cps_p.tile([P, h], f32, tag="cps")
        for jp in range(npair):
            for hh in range(h // 512):
                nc.tensor.matmul(
                    ps[:, hh * 512:(hh + 1) * 512],
                    lhsT=ws[:, 2 * jp:2 * jp + 2, :],
                    rhs=gt[:, 2 * jp:2 * jp + 2, hh * 512:(hh + 1) * 512],
                    start=(jp == 0),
                    stop=(jp == npair - 1),
                    perf_mode=DR,
                )
        agg = agg_p.tile([P, h], bf16, tag="agg")
        nc.vector.tensor_copy(out=agg[:], in_=ps[:])
        for m in range(kh):
            tp = tps_p.tile([P, P], bf16, tag="tps")
            nc.tensor.transpose(out=tp[:], in_=agg[:, m * P:(m + 1) * P], identity=ident[:])
            nc.vector.tensor_copy(out=xb[:, m, :], in_=tp[:])

    with tile.TileContext(nc) as tc:
        with tc.tile_pool(name="consts", bufs=1) as consts:
            s_sb = consts.tile([P, 3 * nb], f32)
            nc.sync.dma_start(out=s_sb[:], in_=s_all[:])
            idx1_sb = consts.tile([P, nb * cpb * 8], i16)
            nc.sync.dma_start(out=idx1_sb[:], in_=idx1[:])
            idx2_sb = consts.tile([P, nb * cpb * 8], i16)
            nc.sync.dma_start(out=idx2_sb[:], in_=idx2[:])
            dcol1_sb = consts.tile([P, nb * cpb], f32)
            nc.sync.dma_start(out=dcol1_sb[:], in_=dcol1[:])
            dcol2_sb = consts.tile([P, nb * cpb], f32)
            nc.sync.dma_start(out=dcol2_sb[:], in_=dcol2[:])
            ident = consts.tile([P, P], bf16)
            make_identity(nc, ident[:])
            # iota_w[p, j, m] = m  (f32; values 0..127 are exact)
            iota_w = consts.tile([P, cpb, P], f32)
            nc.gpsimd.iota(
                iota_w[:], pattern=[[0, cpb], [1, P]], base=0,
                channel_multiplier=0, allow_small_or_imprecise_dtypes=True,
            )

            # ------------- phase 1: x1 = relu(W1^T featT) by 512-col groups;
            # g1[b] = (x1[b] @ Wc1) * s1[b] interleaved per 4-block group
            with tc.tile_pool(name="ph1", bufs=1) as ph1, \
                 tc.tile_pool(name="ft", bufs=2) as ft_p, \
                 tc.tile_pool(name="h1g", bufs=2) as h1g_p, \
                 tc.tile_pool(name="ps1", bufs=4, space="PSUM") as ps1_p, \
                 tc.tile_pool(name="gps1", bufs=1, space="PSUM") as gps1_p, \
                 tc.tile_pool(name="gout", bufs=2) as gout_p:
                w1_sb = ph1.tile([P, ki, h], bf16)
                nc.sync.dma_start(out=w1_sb[:], in_=w1[:].rearrange("p (k n) -> p k n", k=ki))
                wc1_sb = ph1.tile([P, kh, h], bf16)
                nc.sync.dma_start(out=wc1_sb[:], in_=wc1[:].rearrange("p (k n) -> p k n", k=kh))
                featT_r = featT[:].rearrange("p (k n) -> p k n", k=ki)
                for g in range(ngrp):
                    ft = ft_p.tile([P, ki, 512], bf16, tag="ft")
                    nc.sync.dma_start(out=ft[:], in_=featT_r[:, :, g * 512:(g + 1) * 512])
                    h1g = h1g_p.tile([P, kh, 512], bf16, tag="h1g")
                    for m in range(kh):
                        ps = ps1_p.tile([P, 512], f32, tag="ps1")
                        for k in range(ki):
                            nc.tensor.matmul(
                                ps[:],
                                lhsT=w1_sb[:, k, m * P:(m + 1) * P],
                                rhs=ft[:, k, :],
                                start=(k == 0),
                                stop=(k == ki - 1),
                            )
                        nc.scalar.activation(out=h1g[:, m, :], in_=ps[:], func=Relu)
                    for bq in range(4):
                        b = g * 4 + bq
                        ps2 = gps1_p.tile([P, h], f32, tag="gps")
                        for k in range(kh):
                            for hh in range(h // 512):
                                nc.tensor.matmul(
                                    ps2[:, hh * 512:(hh + 1) * 512],
                                    lhsT=h1g[:, k, bq * P:(bq + 1) * P],
                                    rhs=wc1_sb[:, k, hh * 512:(hh + 1) * 512],
                                    start=(k == 0),
                                    stop=(k == kh - 1),
                                )
                        gsb = gout_p.tile([P, h], f8, tag="gsb")
                        nc.vector.tensor_scalar(
                            out=gsb[:], in0=ps2[:], scalar1=s_sb[:, b:b + 1],
                            scalar2=None, op0=mult,
                        )
                        dst_t = ag_in[1, "a"] if b < nhb else ag_in[1, "b"]
                        roff = (b if b < nhb else b - nhb) * P
                        nc.sync.dma_start(out=dst_t[roff:roff + P, :], in_=gsb[:])

            # half-a collective fires as soon as blocks 0..9 are done
            for hf in ("a", "b"):
                nc.gpsimd.collective_compute(
                    "AllGather", mybir.AluOpType.bypass,
                    ins=[ag_in[1, hf][:]], outs=[ag_out[1, hf][:]], replica_groups=rg,
                )

            # ------------- phases 2-4 share the resident W2 tile
            if phases >= 2:
                (ncA1, ncE1), (ncA2, ncE2) = metas
                with tc.tile_pool(name="ph234", bufs=1) as ph234:
                    w2_sb = ph234.tile([P, kh, go], bf16)
                    w2_r = w2[:].rearrange("p (k n) -> p k n", k=kh)
                    for gstart, gn in fgroups:
                        nc.sync.dma_start(
                            out=w2_sb[:, :, gstart:gstart + gn],
                            in_=w2_r[:, :, gstart:gstart + gn],
                        )

                    # ----- phase 2: conv1 per block -> x2[b]; g2[b] = (x2[b] @ Wc2) * s2[b]
                    with tc.tile_pool(name="ph2", bufs=1) as ph2, \
                         tc.tile_pool(name="gat", bufs=2) as gat_p, \
                         tc.tile_pool(name="wsl", bufs=2) as wsl_p, \
                         tc.tile_pool(name="agg", bufs=2) as agg_p, \
                         tc.tile_pool(name="x2b", bufs=3) as x2b_p, \
                         tc.tile_pool(name="gout2", bufs=2) as gout2_p, \
                         tc.tile_pool(name="cps", bufs=2, space="PSUM") as cps_p, \
                         tc.tile_pool(name="tps", bufs=2, space="PSUM") as tps_p, \
                         tc.tile_pool(name="gps2", bufs=1, space="PSUM") as gps2_p:
                        wc2_sb = ph2.tile([P, kh, h], bf16)
                        nc.sync.dma_start(out=wc2_sb[:], in_=wc2[:].rearrange("p (k n) -> p k n", k=kh))
                        for b in range(nb):
                            gt = gat_p.tile([P, cpb, h], f8, tag="gt")
                            gather_half(gt, ag_out[1, "a"], idx1_sb, b, 0, ncA1[b])
                            gather_half(gt, ag_out[1, "b"], idx1_sb, b, ncA1[b], ncE1[b] - ncA1[b])
                            ws = wsl_p.tile([P, cpb, P], f8, tag="ws")
                            build_ws(ws, iota_w, dcol1_sb, b)
                            x2b = x2b_p.tile([P, kh, P], bf16, tag="x2b")
                            conv_block(gt, ws, ident, x2b, ncE1[b], cps_p, tps_p, agg_p)
                            ps2 = gps2_p.tile([P, h], f32, tag="g2ps")
                            for k in range(kh):
                                for hh in range(h // 512):
                                    nc.tensor.matmul(
                                        ps2[:, hh * 512:(hh + 1) * 512],
                                        lhsT=x2b[:, k, :],
                                        rhs=wc2_sb[:, k, hh * 512:(hh + 1) * 512],
                                        start=(k == 0),
                                        stop=(k == kh - 1),
                                    )
                            gsb = gout2_p.tile([P, h], f8, tag="gsb2")
                            nc.vector.tensor_scalar(
                                out=gsb[:], in0=ps2[:], scalar1=s_sb[:, nb + b:nb + b + 1],
                                scalar2=None, op0=mult,
                            )
                            dst_t = ag_in[2, "a"] if b < nhb else ag_in[2, "b"]
                            roff = (b if b < nhb else b - nhb) * P
                            nc.sync.dma_start(out=dst_t[roff:roff + P, :], in_=gsb[:])

                    for hf in ("a", "b"):
                        nc.gpsimd.collective_compute(
                            "AllGather", mybir.AluOpType.bypass,
                            ins=[ag_in[2, hf][:]], outs=[ag_out[2, hf][:]], replica_groups=rg,
                        )

                    # ----- phase 3+4: conv2 per block -> x3[b]; out[b] = sigmoid(s3*(x3[b] @ W2))
                    if phases >= 3:
                        with tc.tile_pool(name="gat3", bufs=2) as gat3_p, \
                             tc.tile_pool(name="wsl3", bufs=2) as wsl3_p, \
                             tc.tile_pool(name="agg3", bufs=2) as agg3_p, \
                             tc.tile_pool(name="x3b", bufs=3) as x3b_p, \
                             tc.tile_pool(name="fout", bufs=3) as fout_p, \
                             tc.tile_pool(name="cps3", bufs=2, space="PSUM") as cps3_p, \
                             tc.tile_pool(name="tps3", bufs=2, space="PSUM") as tps3_p, \
                             tc.tile_pool(name="fps", bufs=2, space="PSUM") as fps_p:
                            for b in range(nb):
                                gt = gat3_p.tile([P, cpb, h], f8, tag="gt3")
                                gather_half(gt, ag_out[2, "a"], idx2_sb, b, 0, ncA2[b])
                                gather_half(gt, ag_out[2, "b"], idx2_sb, b, ncA2[b], ncE2[b] - ncA2[b])
                                ws = wsl3_p.tile([P, cpb, P], f8, tag="ws3")
                                build_ws(ws, iota_w, dcol2_sb, b)
                                x3b = x3b_p.tile([P, kh, P], bf16, tag="x3b")
                                conv_block(gt, ws, ident, x3b, ncE2[b], cps3_p, tps3_p, agg3_p)
                                if phases >= 4:
                                    for gstart, gn in fgroups:
                                        o = fout_p.tile([P, 2048], bf16, tag="fo")
                                        for cs in range(0, gn, 512):
                                            cn = min(512, gn - cs)
                                            ps4 = fps_p.tile([P, 512], f32, tag="fps")
                                            for k in range(kh):
                                                nc.tensor.matmul(
                                                    ps4[:, :cn],
                                                    lhsT=x3b[:, k, :],
                                                    rhs=w2_sb[:, k, gstart + cs:gstart + cs + cn],
                                                    start=(k == 0),
                                                    stop=(k == kh - 1),
                                                )
                                            nc.scalar.activation(
                                                out=o[:, cs:cs + cn], in_=ps4[:, :cn], func=Sigmoid,
                                                scale=s_sb[:, 2 * nb + b:2 * nb + b + 1],
                                            )
                                        nc.sync.dma_start(
                                            out=out_d[b * P:(b + 1) * P, gstart:gstart + gn],
                                            in_=o[:, :gn],
                                        )

    nc.compile()
    return nc


# ---------------------------------------------------------------- entry point

def _ensure_ntff_hook():
    """Register the axon NTFF profile hook if the image's antenv lacks it."""
    import contextlib
    import ctypes
    import sys
    import types

    try:
        from antenv.axon_hooks import get_axon_ntff_profile_hook  # noqa: F401
        return
    except ImportError:
        pass
    try:
        import antenv
    except ImportError:
        return
    mod = types.ModuleType("antenv.axon_hooks")
    holder = [None]
    mod.set_axon_ntff_profile_hook = lambda h: holder.__setitem__(0, h)
    mod.get_axon_ntff_profile_hook = lambda: holder[0]
    sys.modules["antenv.axon_hooks"] = mod
    antenv.axon_hooks = mod
    try:
        lib = ctypes.CDLL("/opt/axon/libaxon_pjrt.so")
    except OSError:
        return
    if not hasattr(lib, "axon_start_nrt_profile"):
        return
    lib.axon_start_nrt_profile.argtypes = [
        ctypes.POINTER(ctypes.c_int64),
        ctypes.c_size_t,
    ]
    lib.axon_start_nrt_profile.restype = ctypes.c_int64
    lib.axon_stop_nrt_profile.argtypes = [ctypes.c_char_p]
    lib.axon_stop_nrt_profile.restype = ctypes.c_int64

    @contextlib.contextmanager
    def _hook(output_dir, device_ids):
        import jax

        jax.devices()
        if device_ids:
            ids = (ctypes.c_int64 * len(device_ids))(*device_ids)
            rc = lib.axon_start_nrt_profile(ids, len(device_ids))
        else:
            rc = lib.axon_start_nrt_profile(None, 0)
        if rc != 0:
            raise RuntimeError(f"axon_start_nrt_profile rc={rc}")
        try:
            yield
        finally:
            n = lib.axon_stop_nrt_profile(str(output_dir).encode())
            print(f"profile: {n} file(s) written to {output_dir}", file=sys.stderr)

    holder[0] = _hook


def _run_hw(cfg, inputs, trace=False):
    if trace:
        _ensure_ntff_hook()
    cpb, metas, in_maps = prep_inputs(cfg, inputs)
    phases = int(os.environ.get("GNN_PHASES", "4"))
    nc = build_bass(cfg, cpb, metas, phases=phases)
    res = run_bass_kernel_spmd(nc, in_maps, core_ids=list(range(cfg.n_cores)), trace=trace)
    full = np.concatenate(
        [np.asarray(res.results[c]["out"]).astype(np.float32) for c in range(cfg.n_cores)],
        axis=0,
    )
    return full[: cfg.n_nodes], res


def kernel(**inputs) -> np.ndarray:
    trace = bool(int(os.environ.get("GNN_TRACE", "0")))
    out, res = _run_hw(FULL, inputs, trace=trace)
    if trace and res.exec_time_ns is not None:
        print(f"HW exec time: {res.exec_time_ns} ns")
    return out


# revision 27
# speedup vs baseline: 1.3077x; 1.0807x over previous
"""Trainium2 Bass kernel for DeepGraphGO-style 2-layer GraphConv model.

  x1 = relu(features @ W1 + b1)
  x2 = GraphConv(x1; src1, dst1, Wc1, bc1)   # D_in^-1/2 A D_out^-1/2 x W + b
  x3 = GraphConv(x2; src2, dst2, Wc2, bc2)
  out = sigmoid(x3 @ W2 + b2)

Sharding: nodes are padded to 20480 and split contiguously across 8 cores
(2560 nodes per core, 20 blocks of 128).  Each core computes its node shard
through every layer; the per-layer "message" tensors g = (x @ Wc) * deg_out^-1/2
are quantized to fp8-e4m3 and AllGathered so every core can gather arbitrary
source rows (1 KB/row).

The per-edge gather is bound by SWDGE descriptor generation on the GpSimd Q7
(~10 ns/row, engine-serial), so the kernel is organized to keep that engine
busy continuously: each layer's AllGather is split into two node-half
collectives (first half fires as soon as the first 10 blocks' messages are
ready, overlapping the producing phase), and each destination block's edges
are host-sorted by (dst, src-half) so gathers for the first half start before
the second collective lands.

The segment-sum is computed per 128-node destination block as one-hot
selection matmuls on the tensor engine in fp8 DoubleRow mode (256 edges per
pass); one-hot matrices are built on-device (iota + is_equal on the vector
engine) from compact per-edge destination-column ids.  The final x3 @ W2 GEMM
is interleaved per block into the conv2 loop so tensor-engine work overlaps
gather DMA; output is written bf16 and upcast on host.
"""

import math
import os
from dataclasses import dataclass

import numpy as np
import ml_dtypes

import concourse.bass as bass
import concourse.bacc as bacc
import concourse.tile as tile
from concourse import mybir
from concourse.masks import make_identity
from concourse.bass_utils import run_bass_kernel_spmd

BF16 = ml_dtypes.bfloat16
FP8 = ml_dtypes.float8_e4m3
P = 128


@dataclass(frozen=True)
class Cfg:
    n_nodes: int = 20000          # real nodes
    n_cores: int = 8
    nb: int = 20                  # 128-node blocks per core
    fin: int = 2048               # input feature dim
    h: int = 1024                 # hidden dim
    go: int = 5000                # output dim

    @property
    def npc(self):                # nodes per core (padded)
        return self.nb * P

    @property
    def nba(self):                # blocks in AG half A (fires early)
        return (self.nb * 3) // 4

    @property
    def nha(self):                # nodes per core in half A
        return self.nba * P

    @property
    def nhb_(self):               # nodes per core in half B
        return self.npc - self.nha

    @property
    def n_pad(self):
        return self.n_cores * self.npc

    @property
    def ki(self):                 # fin 128-chunks
        return self.fin // P

    @property
    def kh(self):                 # h 128-chunks
        return self.h // P


FULL = Cfg()


# ---------------------------------------------------------------- host prep

def _tile_kmaj(w, k_chunks, ncols):
    """[k_chunks*128, ncols] -> [128, k_chunks*ncols] with dev[p, k*ncols+j] = w[k*128+p, j]."""
    return np.ascontiguousarray(
        w.reshape(k_chunks, P, ncols).transpose(1, 0, 2).reshape(P, k_chunks * ncols)
    )


def _edge_prep(cfg, src, dst, cpb=None):
    """Per-core edge structures for one conv layer, with per-destination-block
    edges grouped by source node-half (half A: src%npc < nh).

    Returns (cpb, meta, per_core list of (idx_dev int16 [128, nb*cpb*8],
    dcol_dev f32 [128, nb*cpb])).  meta = (maxA, maxB, ncA, ncB): per-block
    max-over-cores real edge counts per half and chunk counts (shared across
    cores).  Gather row ids index the half buffer: core*nh + (src%npc) - half*nh.
    """
    npc, nb, nha, nhb_ = cfg.npc, cfg.nb, cfg.nha, cfg.nhb_
    per_core = []
    for c in range(cfg.n_cores):
        sel = (dst >= c * npc) & (dst < (c + 1) * npc)
        s_e = src[sel].astype(np.int64)
        d_e = (dst[sel] - c * npc).astype(np.int64)
        half = ((s_e % npc) >= nha).astype(np.int64)
        order = np.lexsort((half, d_e))
        s_e, d_e, half = s_e[order], d_e[order], half[order]
        blk = d_e // P
        cntA = np.bincount(blk[half == 0], minlength=nb)
        cntB = np.bincount(blk[half == 1], minlength=nb)
        per_core.append((s_e, d_e, half, cntA, cntB))

    maxA = [max(int(pc[3][b]) for pc in per_core) for b in range(nb)]
    maxB = [max(int(pc[4][b]) for pc in per_core) for b in range(nb)]
    ncA = [math.ceil(m / P) for m in maxA]
    ncB = [math.ceil(m / P) for m in maxB]
    # per-block gathered chunk count, padded even for DoubleRow pairing; the
    # pad chunks gather row 0 of the B half so every chunk the matmul reads
    # holds finite data (stale NaN x zero one-hot would still poison PSUM)
    ncE = [a + b + (a + b) % 2 for a, b in zip(ncA, ncB)]
    need_cpb = max(ncE)
    if cpb is None:
        cpb = need_cpb
    assert cpb >= need_cpb
    npad = cpb * P

    out = []
    for s_e, d_e, half, cntA, cntB in per_core:
        idx_flat = np.full((nb, npad), -1, np.int64)     # -1: skipped by ucode
        dcol = np.full((nb, npad), -1.0, np.float32)     # -1 pad -> all-zero ws row
        for b in range(nb):
            mb_ = (d_e // P) == b
            for hsel, nchunks, base in ((0, ncA[b], 0), (1, ncE[b] - ncA[b], ncA[b] * P)):
                m = mb_ & (half == hsel)
                cnt = int(m.sum())
                s_h = s_e[m]
                hw_ = nhb_ if hsel else nha
                rows = (s_h // npc) * hw_ + (s_h % npc) - hsel * nha
                idx_flat[b, base:base + cnt] = rows
                idx_flat[b, base + cnt:base + nchunks * P] = 0   # pad: row 0 (finite)
                dcol[b, base:base + cnt] = (d_e[m] - b * P).astype(np.float32)
        # dcol device layout: [128(edge lane), nb*cpb]; dev[p, b*cpb+j] = dcol[b, j*128+p]
        dcol_dev = np.ascontiguousarray(
            dcol.reshape(nb, cpb, P).transpose(2, 0, 1).reshape(P, nb * cpb)
        )
        # idx layout: wrapped into 16 partitions, replicated x8
        x = idx_flat.reshape(nb, cpb * 8, 16).transpose(2, 0, 1).reshape(16, nb * cpb * 8)
        idx_dev = np.ascontiguousarray(np.tile(x, (8, 1))).astype(np.int16)
        out.append((idx_dev, dcol_dev))
    return cpb, (ncA, ncE), out


def prep_inputs(cfg, inputs):
    """Build the SPMD per-core input maps. Returns (cpb, metas, in_maps)."""
    f32 = np.float32
    feats = np.asarray(inputs["features"], f32)
    W1 = np.asarray(inputs["W1"], f32)
    Wc1 = np.asarray(inputs["Wc1"], f32)
    Wc2 = np.asarray(inputs["Wc2"], f32)
    W2 = np.asarray(inputs["W2"], f32)
    for bname in ("b1", "bc1", "bc2", "b2"):
        assert not np.any(np.asarray(inputs[bname])), f"nonzero bias {bname} unsupported"
    src1 = np.asarray(inputs["src1"]).astype(np.int64)
    dst1 = np.asarray(inputs["dst1"]).astype(np.int64)
    src2 = np.asarray(inputs["src2"]).astype(np.int64)
    dst2 = np.asarray(inputs["dst2"]).astype(np.int64)

    npc, nb, n_pad = cfg.npc, cfg.nb, cfg.n_pad

    deg_out1 = np.maximum(np.bincount(src1, minlength=n_pad), 1.0).astype(f32) ** -0.5
    deg_in1 = np.maximum(np.bincount(dst1, minlength=n_pad), 1.0).astype(f32) ** -0.5
    deg_out2 = np.maximum(np.bincount(src2, minlength=n_pad), 1.0).astype(f32) ** -0.5
    deg_in2 = np.maximum(np.bincount(dst2, minlength=n_pad), 1.0).astype(f32) ** -0.5

    featp = np.zeros((n_pad, cfg.fin), f32)
    featp[: cfg.n_nodes] = feats

    w1_dev = _tile_kmaj(W1, cfg.ki, cfg.h).astype(BF16)
    wc1_dev = _tile_kmaj(Wc1, cfg.kh, cfg.h).astype(BF16)
    wc2_dev = _tile_kmaj(Wc2, cfg.kh, cfg.h).astype(BF16)
    w2_dev = _tile_kmaj(W2, cfg.kh, cfg.go).astype(BF16)

    cpb1, m1, e1 = _edge_prep(cfg, src1, dst1)
    cpb2, m2, e2 = _edge_prep(cfg, src2, dst2)
    cpb = max(cpb1, cpb2)
    if cpb1 < cpb:
        _, m1, e1 = _edge_prep(cfg, src1, dst1, cpb)
    if cpb2 < cpb:
        _, m2, e2 = _edge_prep(cfg, src2, dst2, cpb)

    in_maps = []
    for c in range(cfg.n_cores):
        lo, hi = c * npc, (c + 1) * npc
        featT = featp[lo:hi].T  # [fin, npc]
        featT_dev = _tile_kmaj(np.ascontiguousarray(featT), cfg.ki, npc).astype(BF16)
        s1 = deg_out1[lo:hi].reshape(nb, P).T                      # g1 row scale
        s2 = (deg_in1[lo:hi] * deg_out2[lo:hi]).reshape(nb, P).T   # g2 row scale
        s3 = deg_in2[lo:hi].reshape(nb, P).T                       # final scale
        s_all = np.ascontiguousarray(np.concatenate([s1, s2, s3], axis=1)).astype(f32)
        in_maps.append(
            {
                "featT": featT_dev,
                "w1": w1_dev,
                "wc1": wc1_dev,
                "wc2": wc2_dev,
                "w2": w2_dev,
                "s_all": s_all,
                "idx1": e1[c][0],
                "dcol1": e1[c][1],
                "idx2": e2[c][0],
                "dcol2": e2[c][1],
            }
        )
    return cpb, (m1, m2), in_maps


# ---------------------------------------------------------------- device build

def build_bass(cfg, cpb, metas, phases=4):
    f32, bf16, i16 = mybir.dt.float32, mybir.dt.bfloat16, mybir.dt.int16
    f8 = mybir.dt.float8e4
    nb, npc, ki, kh, h, go = cfg.nb, cfg.npc, cfg.ki, cfg.kh, cfg.h, cfg.go
    nha, nhb_, nba = cfg.nha, cfg.nhb_, cfg.nba
    ngrp = npc // 512

    nc = bacc.Bacc("TRN2", target_bir_lowering=False, debug=False, num_devices=cfg.n_cores)

    featT = nc.dram_tensor("featT", [P, ki * npc], bf16, kind="ExternalInput")
    w1 = nc.dram_tensor("w1", [P, ki * h], bf16, kind="ExternalInput")
    wc1 = nc.dram_tensor("wc1", [P, kh * h], bf16, kind="ExternalInput")
    wc2 = nc.dram_tensor("wc2", [P, kh * h], bf16, kind="ExternalInput")
    w2 = nc.dram_tensor("w2", [P, kh * go], bf16, kind="ExternalInput")
    s_all = nc.dram_tensor("s_all", [P, 3 * nb], f32, kind="ExternalInput")
    idx1 = nc.dram_tensor("idx1", [P, nb * cpb * 8], i16, kind="ExternalInput")
    dcol1 = nc.dram_tensor("dcol1", [P, nb * cpb], f32, kind="ExternalInput")
    idx2 = nc.dram_tensor("idx2", [P, nb * cpb * 8], i16, kind="ExternalInput")
    dcol2 = nc.dram_tensor("dcol2", [P, nb * cpb], f32, kind="ExternalInput")
    out_d = nc.dram_tensor("out", [npc, go], bf16, kind="ExternalOutput")

    ag_in = {}
    ag_out = {}
    for layer in (1, 2):
        for hf, hw_ in (("a", nha), ("b", nhb_)):
            ag_in[layer, hf] = nc.dram_tensor(f"ag{layer}{hf}_in", [hw_, h], f8, kind="Internal")
            ag_out[layer, hf] = nc.dram_tensor(
                f"ag{layer}{hf}_out", [cfg.n_cores * hw_, h], f8,
                kind="Internal", addr_space="Shared",
            )

    mult = mybir.AluOpType.mult
    is_eq = mybir.AluOpType.is_equal
    Relu = mybir.ActivationFunctionType.Relu
    Sigmoid = mybir.ActivationFunctionType.Sigmoid
    DR = mybir.MatmulPerfMode.DoubleRow
    rg = [list(range(cfg.n_cores))]

    # final-phase output column groups
    fgroups = []
    gstart = 0
    while gstart < go:
        gn = min(2048, go - gstart)
        fgroups.append((gstart, gn))
        gstart += gn

    def build_ws(ws, iota_w, dcol_sb, b):
        """One-hot scatter matrices for dst block b: ws[p, j, m] = (dcol[p, b*cpb+j] == m)."""
        nc.vector.tensor_tensor(
            out=ws[:], in0=iota_w[:],
            in1=dcol_sb[:, b * cpb:(b + 1) * cpb].broadcast_to([P, cpb, P]),
            op=is_eq,
        )

    gjn = int(os.environ.get("GNN_JN", "9"))

    def gather_half(gt, ag_out_t, idx_sb, b, base, nch):
        """Gather one source-half of block b: chunks [base, base+nch) of gt."""
        for j0 in range(0, nch, gjn):
            jn = min(gjn, nch - j0)
            nc.gpsimd.dma_gather(
                gt[:, base + j0:base + j0 + jn, :].bitcast(bf16),
                ag_out_t[:].bitcast(bf16),
                idx_sb[:, (b * cpb + base + j0) * 8:(b * cpb + base + j0 + jn) * 8],
                jn * P, jn * P, h // 2,
            )

    def conv_block(gt, ws, ident, xb, nc_b, cps_p, tps_p, agg_p):
        """One dst block: DoubleRow scatter matmuls + transpose to feature-major xb."""
        npair = nc_b // 2
        ps = cps_p.tile([P, h], f32, tag="cps")
        for jp in range(npair):
            for hh in range(h // 512):
                nc.tensor.matmul(
                    ps[:, hh * 512:(hh + 1) * 512],
                    lhsT=ws[:, 2 * jp:2 * jp + 2, :],
                    rhs=gt[:, 2 * jp:2 * jp + 2, hh * 512:(hh + 1) * 512],
                    start=(jp == 0),
                    stop=(jp == npair - 1),
                    perf_mode=DR,
                )
        agg = agg_p.tile([P, h], bf16, tag="agg")
        nc.vector.tensor_copy(out=agg[:], in_=ps[:])
        for m in range(kh):
            tp = tps_p.tile([P, P], bf16, tag="tps")
            nc.tensor.transpose(out=tp[:], in_=agg[:, m * P:(m + 1) * P], identity=ident[:])
            nc.vector.tensor_copy(out=xb[:, m, :], in_=tp[:])

    with tile.TileContext(nc) as tc:
        with tc.tile_pool(name="consts", bufs=1) as consts:
            s_sb = consts.tile([P, 3 * nb], f32)
            nc.sync.dma_start(out=s_sb[:], in_=s_all[:])
            idx1_sb = consts.tile([P, nb * cpb * 8], i16)
            nc.sync.dma_start(out=idx1_sb[:], in_=idx1[:])
            idx2_sb = consts.tile([P, nb * cpb * 8], i16)
            nc.sync.dma_start(out=idx2_sb[:], in_=idx2[:])
            dcol1_sb = consts.tile([P, nb * cpb], f32)
            nc.sync.dma_start(out=dcol1_sb[:], in_=dcol1[:])
            dcol2_sb = consts.tile([P, nb * cpb], f32)
            nc.sync.dma_start(out=dcol2_sb[:], in_=dcol2[:])
            ident = consts.tile([P, P], bf16)
            make_identity(nc, ident[:])
            # iota_w[p, j, m] = m  (f32; values 0..127 are exact)
            iota_w = consts.tile([P, cpb, P], f32)
            nc.gpsimd.iota(
                iota_w[:], pattern=[[0, cpb], [1, P]], base=0,
                channel_multiplier=0, allow_small_or_imprecise_dtypes=True,
            )

            # ------------- phase 1: x1 = relu(W1^T featT) by 512-col groups;
            # g1[b] = (x1[b] @ Wc1) * s1[b] interleaved per 4-block group
            with tc.tile_pool(name="ph1", bufs=1) as ph1, \
                 tc.tile_pool(name="ft", bufs=2) as ft_p, \
                 tc.tile_pool(name="h1g", bufs=2) as h1g_p, \
                 tc.tile_pool(name="ps1", bufs=4, space="PSUM") as ps1_p, \
                 tc.tile_pool(name="gps1", bufs=1, space="PSUM") as gps1_p, \
                 tc.tile_pool(name="gout", bufs=2) as gout_p:
                w1_sb = ph1.tile([P, ki, h], bf16)
                nc.sync.dma_start(out=w1_sb[:], in_=w1[:].rearrange("p (k n) -> p k n", k=ki))
                wc1_sb = ph1.tile([P, kh, h], bf16)
                nc.sync.dma_start(out=wc1_sb[:], in_=wc1[:].rearrange("p (k n) -> p k n", k=kh))
                featT_r = featT[:].rearrange("p (k n) -> p k n", k=ki)
                for g in range(ngrp):
                    ft = ft_p.tile([P, ki, 512], bf16, tag="ft")
                    nc.sync.dma_start(out=ft[:], in_=featT_r[:, :, g * 512:(g + 1) * 512])
                    h1g = h1g_p.tile([P, kh, 512], bf16, tag="h1g")
                    for m in range(kh):
                        ps = ps1_p.tile([P, 512], f32, tag="ps1")
                        for k in range(ki):
                            nc.tensor.matmul(
                                ps[:],
                                lhsT=w1_sb[:, k, m * P:(m + 1) * P],
                                rhs=ft[:, k, :],
                                start=(k == 0),
                                stop=(k == ki - 1),
                            )
                        nc.scalar.activation(out=h1g[:, m, :], in_=ps[:], func=Relu)
                    for bq in range(4):
                        b = g * 4 + bq
                        ps2 = gps1_p.tile([P, h], f32, tag="gps")
                        for k in range(kh):
                            for hh in range(h // 512):
                                nc.tensor.matmul(
                                    ps2[:, hh * 512:(hh + 1) * 512],
                                    lhsT=h1g[:, k, bq * P:(bq + 1) * P],
                                    rhs=wc1_sb[:, k, hh * 512:(hh + 1) * 512],
                                    start=(k == 0),
                                    stop=(k == kh - 1),
                                )
                        gsb = gout_p.tile([P, h], f8, tag="gsb")
                        nc.vector.tensor_scalar(
                            out=gsb[:], in0=ps2[:], scalar1=s_sb[:, b:b + 1],
                            scalar2=None, op0=mult,
                        )
                        dst_t = ag_in[1, "a"] if b < nba else ag_in[1, "b"]
                        roff = (b if b < nba else b - nba) * P
                        nc.sync.dma_start(out=dst_t[roff:roff + P, :], in_=gsb[:])

            # half-a collective fires as soon as blocks 0..9 are done
            for hf in ("a", "b"):
                nc.gpsimd.collective_compute(
                    "AllGather", mybir.AluOpType.bypass,
                    ins=[ag_in[1, hf][:]], outs=[ag_out[1, hf][:]], replica_groups=rg,
                )

            # ------------- phases 2-4 share the resident W2 tile
            if phases >= 2:
                (ncA1, ncE1), (ncA2, ncE2) = metas
                with tc.tile_pool(name="ph234", bufs=1) as ph234:
                    w2_sb = ph234.tile([P, kh, go], bf16)
                    w2_r = w2[:].rearrange("p (k n) -> p k n", k=kh)
                    for gstart, gn in fgroups:
                        nc.sync.dma_start(
                            out=w2_sb[:, :, gstart:gstart + gn],
                            in_=w2_r[:, :, gstart:gstart + gn],
                        )

                    # ----- phase 2: conv1 per block -> x2[b]; g2[b] = (x2[b] @ Wc2) * s2[b]
                    with tc.tile_pool(name="ph2", bufs=1) as ph2, \
                         tc.tile_pool(name="gat", bufs=2) as gat_p, \
                         tc.tile_pool(name="wsl", bufs=2) as wsl_p, \
                         tc.tile_pool(name="agg", bufs=2) as agg_p, \
                         tc.tile_pool(name="x2b", bufs=3) as x2b_p, \
                         tc.tile_pool(name="gout2", bufs=2) as gout2_p, \
                         tc.tile_pool(name="cps", bufs=2, space="PSUM") as cps_p, \
                         tc.tile_pool(name="tps", bufs=2, space="PSUM") as tps_p, \
                         tc.tile_pool(name="gps2", bufs=1, space="PSUM") as gps2_p:
                        wc2_sb = ph2.tile([P, kh, h], bf16)
                        nc.sync.dma_start(out=wc2_sb[:], in_=wc2[:].rearrange("p (k n) -> p k n", k=kh))
                        for b in range(nb):
                            gt = gat_p.tile([P, cpb, h], f8, tag="gt")
                            gather_half(gt, ag_out[1, "a"], idx1_sb, b, 0, ncA1[b])
                            gather_half(gt, ag_out[1, "b"], idx1_sb, b, ncA1[b], ncE1[b] - ncA1[b])
                            ws = wsl_p.tile([P, cpb, P], f8, tag="ws")
                            build_ws(ws, iota_w, dcol1_sb, b)
                            x2b = x2b_p.tile([P, kh, P], bf16, tag="x2b")
                            conv_block(gt, ws, ident, x2b, ncE1[b], cps_p, tps_p, agg_p)
                            ps2 = gps2_p.tile([P, h], f32, tag="g2ps")
                            for k in range(kh):
                                for hh in range(h // 512):
                                    nc.tensor.matmul(
                                        ps2[:, hh * 512:(hh + 1) * 512],
                                        lhsT=x2b[:, k, :],
                                        rhs=wc2_sb[:, k, hh * 512:(hh + 1) * 512],
                                        start=(k == 0),
                                        stop=(k == kh - 1),
                                    )
                            gsb = gout2_p.tile([P, h], f8, tag="gsb2")
                            nc.vector.tensor_scalar(
                                out=gsb[:], in0=ps2[:], scalar1=s_sb[:, nb + b:nb + b + 1],
                                scalar2=None, op0=mult,
                            )
                            dst_t = ag_in[2, "a"] if b < nba else ag_in[2, "b"]
                            roff = (b if b < nba else b - nba) * P
                            nc.sync.dma_start(out=dst_t[roff:roff + P, :], in_=gsb[:])

                    for hf in ("a", "b"):
                        nc.gpsimd.collective_compute(
                            "AllGather", mybir.AluOpType.bypass,
                            ins=[ag_in[2, hf][:]], outs=[ag_out[2, hf][:]], replica_groups=rg,
                        )

                    # ----- phase 3+4: conv2 per block -> x3[b]; out[b] = sigmoid(s3*(x3[b] @ W2))
                    if phases >= 3:
                        with tc.tile_pool(name="gat3", bufs=2) as gat3_p, \
                             tc.tile_pool(name="wsl3", bufs=2) as wsl3_p, \
                             tc.tile_pool(name="agg3", bufs=2) as agg3_p, \
                             tc.tile_pool(name="x3b", bufs=3) as x3b_p, \
                             tc.tile_pool(name="fout", bufs=3) as fout_p, \
                             tc.tile_pool(name="cps3", bufs=2, space="PSUM") as cps3_p, \
                             tc.tile_pool(name="tps3", bufs=2, space="PSUM") as tps3_p, \
                             tc.tile_pool(name="fps", bufs=2, space="PSUM") as fps_p:
                            for b in range(nb):
                                gt = gat3_p.tile([P, cpb, h], f8, tag="gt3")
                                gather_half(gt, ag_out[2, "a"], idx2_sb, b, 0, ncA2[b])
                                gather_half(gt, ag_out[2, "b"], idx2_sb, b, ncA2[b], ncE2[b] - ncA2[b])
                                ws = wsl3_p.tile([P, cpb, P], f8, tag="ws3")
                                build_ws(ws, iota_w, dcol2_sb, b)
                                x3b = x3b_p.tile([P, kh, P], bf16, tag="x3b")
                                conv_block(gt, ws, ident, x3b, ncE2[b], cps3_p, tps3_p, agg3_p)
                                if phases >= 4:
                                    for gstart, gn in fgroups:
                                        o = fout_p.tile([P, 2048], bf16, tag="fo")
                                        for cs in range(0, gn, 512):
                                            cn = min(512, gn - cs)
                                            ps4 = fps_p.tile([P, 512], f32, tag="fps")
                                            for k in range(kh):
                                                nc.tensor.matmul(
                                                    ps4[:, :cn],
                                                    lhsT=x3b[:, k, :],
                                                    rhs=w2_sb[:, k, gstart + cs:gstart + cs + cn],
                                                    start=(k == 0),
                                                    stop=(k == kh - 1),
                                                )
                                            nc.scalar.activation(
                                                out=o[:, cs:cs + cn], in_=ps4[:, :cn], func=Sigmoid,
                                                scale=s_sb[:, 2 * nb + b:2 * nb + b + 1],
                                            )
                                        nc.sync.dma_start(
                                            out=out_d[b * P:(b + 1) * P, gstart:gstart + gn],
                                            in_=o[:, :gn],
                                        )

    nc.compile()
    return nc


# ---------------------------------------------------------------- entry point

def _ensure_ntff_hook():
    """Register the axon NTFF profile hook if the image's antenv lacks it."""
    import contextlib
    import ctypes
    import sys
    import types

    try:
        from antenv.axon_hooks import get_axon_ntff_profile_hook  # noqa: F401
        return
    except ImportError:
        pass
    try:
        import antenv
    except ImportError:
        return
    mod = types.ModuleType("antenv.axon_hooks")
    holder = [None]
    mod.set_axon_ntff_profile_hook = lambda h: holder.__setitem__(0, h)
    mod.get_axon_ntff_profile_hook = lambda: holder[0]
    sys.modules["antenv.axon_hooks"] = mod
    antenv.axon_hooks = mod
    try:
        lib = ctypes.CDLL("/opt/axon/libaxon_pjrt.so")
    except OSError:
        return
    if not hasattr(lib, "axon_start_nrt_profile"):
        return
    lib.axon_start_nrt_profile.argtypes = [
        ctypes.POINTER(ctypes.c_int64),
        ctypes.c_size_t,
    ]
    lib.axon_start_nrt_profile.restype = ctypes.c_int64
    lib.axon_stop_nrt_profile.argtypes = [ctypes.c_char_p]
    lib.axon_stop_nrt_profile.restype = ctypes.c_int64

    @contextlib.contextmanager
    def _hook(output_dir, device_ids):
        import jax

        jax.devices()
        if device_ids:
            ids = (ctypes.c_int64 * len(device_ids))(*device_ids)
            rc = lib.axon_start_nrt_profile(ids, len(device_ids))
        else:
            rc = lib.axon_start_nrt_profile(None, 0)
        if rc != 0:
            raise RuntimeError(f"axon_start_nrt_profile rc={rc}")
        try:
            yield
        finally:
            n = lib.axon_stop_nrt_profile(str(output_dir).encode())
            print(f"profile: {n} file(s) written to {output_dir}", file=sys.stderr)

    holder[0] = _hook


def _run_hw(cfg, inputs, trace=False):
    if trace:
        _ensure_ntff_hook()
    cpb, metas, in_maps = prep_inputs(cfg, inputs)
    phases = int(os.environ.get("GNN_PHASES", "4"))
    nc = build_bass(cfg, cpb, metas, phases=phases)
    res = run_bass_kernel_spmd(nc, in_maps, core_ids=list(range(cfg.n_cores)), trace=trace)
    full = np.concatenate(
        [np.asarray(res.results[c]["out"]).astype(np.float32) for c in range(cfg.n_cores)],
        axis=0,
    )
    return full[: cfg.n_nodes], res


def kernel(**inputs) -> np.ndarray:
    trace = bool(int(os.environ.get("GNN_TRACE", "0")))
    out, res = _run_hw(FULL, inputs, trace=trace)
    if trace and res.exec_time_ns is not None:
        print(f"HW exec time: {res.exec_time_ns} ns")
    return out


# revision 28
# speedup vs baseline: 1.5061x; 1.1518x over previous
"""Trainium2 Bass kernel for DeepGraphGO-style 2-layer GraphConv model.

  x1 = relu(features @ W1 + b1)
  x2 = GraphConv(x1; src1, dst1, Wc1, bc1)   # D_in^-1/2 A D_out^-1/2 x W + b
  x3 = GraphConv(x2; src2, dst2, Wc2, bc2)
  out = sigmoid(x3 @ W2 + b2)

Sharding: nodes are padded to 20480 and split contiguously across 8 cores
(2560 nodes per core, 20 blocks of 128).  Each core computes its node shard
through every layer; the per-layer "message" tensors g = (x @ Wc) * deg_out^-1/2
are quantized to fp8-e4m3 and AllGathered so every core can gather arbitrary
source rows (1 KB/row).

The per-edge gather is bound by SWDGE descriptor generation on the GpSimd Q7
(~10 ns/row, engine-serial), so the kernel is organized to keep that engine
busy continuously: each layer's AllGather is split into two node-half
collectives (first half fires as soon as the first 10 blocks' messages are
ready, overlapping the producing phase), and each destination block's edges
are host-sorted by (dst, src-half) so gathers for the first half start before
the second collective lands.

The segment-sum is computed per 128-node destination block as one-hot
selection matmuls on the tensor engine in fp8 DoubleRow mode (256 edges per
pass); one-hot matrices are built on-device (iota + is_equal on the vector
engine) from compact per-edge destination-column ids.  The final x3 @ W2 GEMM
is interleaved per block into the conv2 loop so tensor-engine work overlaps
gather DMA; output is written bf16 and upcast on host.
"""

import math
import os
from dataclasses import dataclass

import numpy as np
import ml_dtypes

import concourse.bass as bass
import concourse.bacc as bacc
import concourse.tile as tile
from concourse import mybir
from concourse.masks import make_identity
from concourse.bass_utils import run_bass_kernel_spmd

BF16 = ml_dtypes.bfloat16
FP8 = ml_dtypes.float8_e4m3
P = 128


@dataclass(frozen=True)
class Cfg:
    n_nodes: int = 20000          # real nodes
    n_cores: int = 8
    nb: int = 20                  # 128-node blocks per core
    fin: int = 2048               # input feature dim
    h: int = 1024                 # hidden dim
    go: int = 5000                # output dim

    @property
    def npc(self):                # nodes per core (padded)
        return self.nb * P

    @property
    def nba(self):                # blocks in AG half A (fires early)
        return (self.nb * 3) // 4

    @property
    def nha(self):                # nodes per core in half A
        return self.nba * P

    @property
    def nhb_(self):               # nodes per core in half B
        return self.npc - self.nha

    @property
    def n_pad(self):
        return self.n_cores * self.npc

    @property
    def ki(self):                 # fin 128-chunks
        return self.fin // P

    @property
    def kh(self):                 # h 128-chunks
        return self.h // P


FULL = Cfg()


# ---------------------------------------------------------------- host prep

def _tile_kmaj(w, k_chunks, ncols):
    """[k_chunks*128, ncols] -> [128, k_chunks*ncols] with dev[p, k*ncols+j] = w[k*128+p, j]."""
    return np.ascontiguousarray(
        w.reshape(k_chunks, P, ncols).transpose(1, 0, 2).reshape(P, k_chunks * ncols)
    )


def _edge_prep(cfg, src, dst, cpb=None):
    """Per-core edge structures for one conv layer, with per-destination-block
    edges grouped by source node-half (half A: src%npc < nh).

    Returns (cpb, meta, per_core list of (idx_dev int16 [128, nb*cpb*8],
    dcol_dev f32 [128, nb*cpb])).  meta = (maxA, maxB, ncA, ncB): per-block
    max-over-cores real edge counts per half and chunk counts (shared across
    cores).  Gather row ids index the half buffer: core*nh + (src%npc) - half*nh.
    """
    npc, nb, nha, nhb_ = cfg.npc, cfg.nb, cfg.nha, cfg.nhb_
    per_core = []
    for c in range(cfg.n_cores):
        sel = (dst >= c * npc) & (dst < (c + 1) * npc)
        s_e = src[sel].astype(np.int64)
        d_e = (dst[sel] - c * npc).astype(np.int64)
        half = ((s_e % npc) >= nha).astype(np.int64)
        order = np.lexsort((half, d_e))
        s_e, d_e, half = s_e[order], d_e[order], half[order]
        blk = d_e // P
        cntA = np.bincount(blk[half == 0], minlength=nb)
        cntB = np.bincount(blk[half == 1], minlength=nb)
        per_core.append((s_e, d_e, half, cntA, cntB))

    maxA = [max(int(pc[3][b]) for pc in per_core) for b in range(nb)]
    maxB = [max(int(pc[4][b]) for pc in per_core) for b in range(nb)]
    ncA = [math.ceil(m / P) for m in maxA]
    ncB = [math.ceil(m / P) for m in maxB]
    # per-block gathered chunk count, padded even for DoubleRow pairing; the
    # pad chunks gather row 0 of the B half so every chunk the matmul reads
    # holds finite data (stale NaN x zero one-hot would still poison PSUM)
    ncE = [a + b + (a + b) % 2 for a, b in zip(ncA, ncB)]
    need_cpb = max(ncE)
    if cpb is None:
        cpb = need_cpb
    assert cpb >= need_cpb
    npad = cpb * P

    out = []
    for s_e, d_e, half, cntA, cntB in per_core:
        idx_flat = np.full((nb, npad), -1, np.int64)     # -1: skipped by ucode
        dcol = np.full((nb, npad), -1.0, np.float32)     # -1 pad -> all-zero ws row
        for b in range(nb):
            mb_ = (d_e // P) == b
            for hsel, nchunks, base in ((0, ncA[b], 0), (1, ncE[b] - ncA[b], ncA[b] * P)):
                m = mb_ & (half == hsel)
                cnt = int(m.sum())
                s_h = s_e[m]
                hw_ = nhb_ if hsel else nha
                rows = (s_h // npc) * hw_ + (s_h % npc) - hsel * nha
                idx_flat[b, base:base + cnt] = rows
                idx_flat[b, base + cnt:base + nchunks * P] = 0   # pad: row 0 (finite)
                dcol[b, base:base + cnt] = (d_e[m] - b * P).astype(np.float32)
        # dcol device layout: [128(edge lane), nb*cpb]; dev[p, b*cpb+j] = dcol[b, j*128+p]
        dcol_dev = np.ascontiguousarray(
            dcol.reshape(nb, cpb, P).transpose(2, 0, 1).reshape(P, nb * cpb)
        )
        # idx layout: wrapped into 16 partitions, replicated x8
        x = idx_flat.reshape(nb, cpb * 8, 16).transpose(2, 0, 1).reshape(16, nb * cpb * 8)
        idx_dev = np.ascontiguousarray(np.tile(x, (8, 1))).astype(np.int16)
        out.append((idx_dev, dcol_dev))
    return cpb, (ncA, ncE), out


def prep_inputs(cfg, inputs):
    """Build the SPMD per-core input maps. Returns (cpb, metas, in_maps)."""
    f32 = np.float32
    feats = np.asarray(inputs["features"], f32)
    W1 = np.asarray(inputs["W1"], f32)
    Wc1 = np.asarray(inputs["Wc1"], f32)
    Wc2 = np.asarray(inputs["Wc2"], f32)
    W2 = np.asarray(inputs["W2"], f32)
    for bname in ("b1", "bc1", "bc2", "b2"):
        assert not np.any(np.asarray(inputs[bname])), f"nonzero bias {bname} unsupported"
    src1 = np.asarray(inputs["src1"]).astype(np.int64)
    dst1 = np.asarray(inputs["dst1"]).astype(np.int64)
    src2 = np.asarray(inputs["src2"]).astype(np.int64)
    dst2 = np.asarray(inputs["dst2"]).astype(np.int64)

    npc, nb, n_pad = cfg.npc, cfg.nb, cfg.n_pad

    deg_out1 = np.maximum(np.bincount(src1, minlength=n_pad), 1.0).astype(f32) ** -0.5
    deg_in1 = np.maximum(np.bincount(dst1, minlength=n_pad), 1.0).astype(f32) ** -0.5
    deg_out2 = np.maximum(np.bincount(src2, minlength=n_pad), 1.0).astype(f32) ** -0.5
    deg_in2 = np.maximum(np.bincount(dst2, minlength=n_pad), 1.0).astype(f32) ** -0.5

    featp = np.zeros((n_pad, cfg.fin), f32)
    featp[: cfg.n_nodes] = feats

    w1_dev = _tile_kmaj(W1, cfg.ki, cfg.h).astype(BF16)
    wc1_dev = _tile_kmaj(Wc1, cfg.kh, cfg.h).astype(BF16)
    wc2_dev = _tile_kmaj(Wc2, cfg.kh, cfg.h).astype(BF16)
    w2_dev = _tile_kmaj(W2, cfg.kh, cfg.go).astype(FP8)

    cpb1, m1, e1 = _edge_prep(cfg, src1, dst1)
    cpb2, m2, e2 = _edge_prep(cfg, src2, dst2)
    cpb = max(cpb1, cpb2)
    if cpb1 < cpb:
        _, m1, e1 = _edge_prep(cfg, src1, dst1, cpb)
    if cpb2 < cpb:
        _, m2, e2 = _edge_prep(cfg, src2, dst2, cpb)

    in_maps = []
    for c in range(cfg.n_cores):
        lo, hi = c * npc, (c + 1) * npc
        featT = featp[lo:hi].T  # [fin, npc]
        featT_dev = _tile_kmaj(np.ascontiguousarray(featT), cfg.ki, npc).astype(BF16)
        s1 = deg_out1[lo:hi].reshape(nb, P).T                      # g1 row scale
        s2 = (deg_in1[lo:hi] * deg_out2[lo:hi]).reshape(nb, P).T   # g2 row scale
        s3 = deg_in2[lo:hi].reshape(nb, P).T                       # final scale
        s_all = np.ascontiguousarray(np.concatenate([s1, s2, s3], axis=1)).astype(f32)
        in_maps.append(
            {
                "featT": featT_dev,
                "w1": w1_dev,
                "wc1": wc1_dev,
                "wc2": wc2_dev,
                "w2": w2_dev,
                "s_all": s_all,
                "idx1": e1[c][0],
                "dcol1": e1[c][1],
                "idx2": e2[c][0],
                "dcol2": e2[c][1],
            }
        )
    return cpb, (m1, m2), in_maps


# ---------------------------------------------------------------- device build

def build_bass(cfg, cpb, metas, phases=4):
    f32, bf16, i16 = mybir.dt.float32, mybir.dt.bfloat16, mybir.dt.int16
    f8 = mybir.dt.float8e4
    nb, npc, ki, kh, h, go = cfg.nb, cfg.npc, cfg.ki, cfg.kh, cfg.h, cfg.go
    nha, nhb_, nba = cfg.nha, cfg.nhb_, cfg.nba
    ngrp = npc // 512

    nc = bacc.Bacc("TRN2", target_bir_lowering=False, debug=False, num_devices=cfg.n_cores)

    featT = nc.dram_tensor("featT", [P, ki * npc], bf16, kind="ExternalInput")
    w1 = nc.dram_tensor("w1", [P, ki * h], bf16, kind="ExternalInput")
    wc1 = nc.dram_tensor("wc1", [P, kh * h], bf16, kind="ExternalInput")
    wc2 = nc.dram_tensor("wc2", [P, kh * h], bf16, kind="ExternalInput")
    w2 = nc.dram_tensor("w2", [P, kh * go], f8, kind="ExternalInput")
    s_all = nc.dram_tensor("s_all", [P, 3 * nb], f32, kind="ExternalInput")
    idx1 = nc.dram_tensor("idx1", [P, nb * cpb * 8], i16, kind="ExternalInput")
    dcol1 = nc.dram_tensor("dcol1", [P, nb * cpb], f32, kind="ExternalInput")
    idx2 = nc.dram_tensor("idx2", [P, nb * cpb * 8], i16, kind="ExternalInput")
    dcol2 = nc.dram_tensor("dcol2", [P, nb * cpb], f32, kind="ExternalInput")
    out_d = nc.dram_tensor("out", [npc, go], bf16, kind="ExternalOutput")

    ag_in = {}
    ag_out = {}
    for layer in (1, 2):
        for hf, hw_ in (("a", nha), ("b", nhb_)):
            ag_in[layer, hf] = nc.dram_tensor(f"ag{layer}{hf}_in", [hw_, h], f8, kind="Internal")
            ag_out[layer, hf] = nc.dram_tensor(
                f"ag{layer}{hf}_out", [cfg.n_cores * hw_, h], f8,
                kind="Internal", addr_space="Shared",
            )

    mult = mybir.AluOpType.mult
    is_eq = mybir.AluOpType.is_equal
    Relu = mybir.ActivationFunctionType.Relu
    Sigmoid = mybir.ActivationFunctionType.Sigmoid
    DR = mybir.MatmulPerfMode.DoubleRow
    rg = [list(range(cfg.n_cores))]

    # final-phase output column groups
    fgroups = []
    gstart = 0
    while gstart < go:
        gn = min(2048, go - gstart)
        fgroups.append((gstart, gn))
        gstart += gn

    def build_ws(ws, iota_w, dcol_sb, b):
        """One-hot scatter matrices for dst block b: ws[p, j, m] = (dcol[p, b*cpb+j] == m)."""
        nc.vector.tensor_tensor(
            out=ws[:], in0=iota_w[:],
            in1=dcol_sb[:, b * cpb:(b + 1) * cpb].broadcast_to([P, cpb, P]),
            op=is_eq,
        )

    gjn = int(os.environ.get("GNN_JN", "8"))

    def gather_half(gt, ag_out_t, idx_sb, b, base, nch):
        """Gather one source-half of block b: chunks [base, base+nch) of gt."""
        for j0 in range(0, nch, gjn):
            jn = min(gjn, nch - j0)
            nc.gpsimd.dma_gather(
                gt[:, base + j0:base + j0 + jn, :].bitcast(bf16),
                ag_out_t[:].bitcast(bf16),
                idx_sb[:, (b * cpb + base + j0) * 8:(b * cpb + base + j0 + jn) * 8],
                jn * P, jn * P, h // 2,
            )

    def conv_block(gt, ws, ident, xb, nc_b, cps_p, tps_p, agg_p):
        """One dst block: DoubleRow scatter matmuls + transpose to feature-major xb."""
        npair = nc_b // 2
        ps = cps_p.tile([P, h], f32, tag="cps")
        for jp in range(npair):
            for hh in range(h // 512):
                nc.tensor.matmul(
                    ps[:, hh * 512:(hh + 1) * 512],
                    lhsT=ws[:, 2 * jp:2 * jp + 2, :],
                    rhs=gt[:, 2 * jp:2 * jp + 2, hh * 512:(hh + 1) * 512],
                    start=(jp == 0),
                    stop=(jp == npair - 1),
                    perf_mode=DR,
                )
        agg = agg_p.tile([P, h], bf16, tag="agg")
        nc.vector.tensor_copy(out=agg[:], in_=ps[:])
        for m in range(kh):
            tp = tps_p.tile([P, P], bf16, tag="tps")
            nc.tensor.transpose(out=tp[:], in_=agg[:, m * P:(m + 1) * P], identity=ident[:])
            nc.vector.tensor_copy(out=xb[:, m, :], in_=tp[:])

    with tile.TileContext(nc) as tc:
        with tc.tile_pool(name="consts", bufs=1) as consts:
            s_sb = consts.tile([P, 3 * nb], f32)
            nc.sync.dma_start(out=s_sb[:], in_=s_all[:])
            idx1_sb = consts.tile([P, nb * cpb * 8], i16)
            nc.sync.dma_start(out=idx1_sb[:], in_=idx1[:])
            idx2_sb = consts.tile([P, nb * cpb * 8], i16)
            nc.sync.dma_start(out=idx2_sb[:], in_=idx2[:])
            dcol1_sb = consts.tile([P, nb * cpb], f32)
            nc.sync.dma_start(out=dcol1_sb[:], in_=dcol1[:])
            dcol2_sb = consts.tile([P, nb * cpb], f32)
            nc.sync.dma_start(out=dcol2_sb[:], in_=dcol2[:])
            ident = consts.tile([P, P], bf16)
            make_identity(nc, ident[:])
            # iota_w[p, j, m] = m  (f32; values 0..127 are exact)
            iota_w = consts.tile([P, cpb, P], f32)
            nc.gpsimd.iota(
                iota_w[:], pattern=[[0, cpb], [1, P]], base=0,
                channel_multiplier=0, allow_small_or_imprecise_dtypes=True,
            )

            # ------------- phase 1: x1 = relu(W1^T featT) by 512-col groups;
            # g1[b] = (x1[b] @ Wc1) * s1[b] interleaved per 4-block group
            with tc.tile_pool(name="ph1", bufs=1) as ph1, \
                 tc.tile_pool(name="ft", bufs=2) as ft_p, \
                 tc.tile_pool(name="h1g", bufs=2) as h1g_p, \
                 tc.tile_pool(name="ps1", bufs=4, space="PSUM") as ps1_p, \
                 tc.tile_pool(name="gps1", bufs=1, space="PSUM") as gps1_p, \
                 tc.tile_pool(name="gout", bufs=2) as gout_p:
                w1_sb = ph1.tile([P, ki, h], bf16)
                nc.sync.dma_start(out=w1_sb[:], in_=w1[:].rearrange("p (k n) -> p k n", k=ki))
                wc1_sb = ph1.tile([P, kh, h], bf16)
                nc.sync.dma_start(out=wc1_sb[:], in_=wc1[:].rearrange("p (k n) -> p k n", k=kh))
                featT_r = featT[:].rearrange("p (k n) -> p k n", k=ki)
                for g in range(ngrp):
                    ft = ft_p.tile([P, ki, 512], bf16, tag="ft")
                    nc.sync.dma_start(out=ft[:], in_=featT_r[:, :, g * 512:(g + 1) * 512])
                    h1g = h1g_p.tile([P, kh, 512], bf16, tag="h1g")
                    for m in range(kh):
                        ps = ps1_p.tile([P, 512], f32, tag="ps1")
                        for k in range(ki):
                            nc.tensor.matmul(
                                ps[:],
                                lhsT=w1_sb[:, k, m * P:(m + 1) * P],
                                rhs=ft[:, k, :],
                                start=(k == 0),
                                stop=(k == ki - 1),
                            )
                        nc.scalar.activation(out=h1g[:, m, :], in_=ps[:], func=Relu)
                    for bq in range(4):
                        b = g * 4 + bq
                        ps2 = gps1_p.tile([P, h], f32, tag="gps")
                        for k in range(kh):
                            for hh in range(h // 512):
                                nc.tensor.matmul(
                                    ps2[:, hh * 512:(hh + 1) * 512],
                                    lhsT=h1g[:, k, bq * P:(bq + 1) * P],
                                    rhs=wc1_sb[:, k, hh * 512:(hh + 1) * 512],
                                    start=(k == 0),
                                    stop=(k == kh - 1),
                                )
                        gsb = gout_p.tile([P, h], f8, tag="gsb")
                        nc.vector.tensor_scalar(
                            out=gsb[:], in0=ps2[:], scalar1=s_sb[:, b:b + 1],
                            scalar2=None, op0=mult,
                        )
                        dst_t = ag_in[1, "a"] if b < nba else ag_in[1, "b"]
                        roff = (b if b < nba else b - nba) * P
                        nc.sync.dma_start(out=dst_t[roff:roff + P, :], in_=gsb[:])

            # half-a collective fires as soon as blocks 0..9 are done
            for hf in ("a", "b"):
                nc.gpsimd.collective_compute(
                    "AllGather", mybir.AluOpType.bypass,
                    ins=[ag_in[1, hf][:]], outs=[ag_out[1, hf][:]], replica_groups=rg,
                )

            # ------------- phases 2-4 share the resident W2 tile
            if phases >= 2:
                (ncA1, ncE1), (ncA2, ncE2) = metas
                with tc.tile_pool(name="ph234", bufs=1) as ph234:
                    w2_sb = ph234.tile([P, kh, go], f8)
                    w2_r = w2[:].rearrange("p (k n) -> p k n", k=kh)
                    for gstart, gn in fgroups:
                        nc.sync.dma_start(
                            out=w2_sb[:, :, gstart:gstart + gn],
                            in_=w2_r[:, :, gstart:gstart + gn],
                        )

                    # ----- phase 2: conv1 per block -> x2[b]; g2[b] = (x2[b] @ Wc2) * s2[b]
                    with tc.tile_pool(name="ph2", bufs=1) as ph2, \
                         tc.tile_pool(name="gat", bufs=2) as gat_p, \
                         tc.tile_pool(name="wsl", bufs=2) as wsl_p, \
                         tc.tile_pool(name="agg", bufs=2) as agg_p, \
                         tc.tile_pool(name="x2b", bufs=3) as x2b_p, \
                         tc.tile_pool(name="gout2", bufs=2) as gout2_p, \
                         tc.tile_pool(name="cps", bufs=2, space="PSUM") as cps_p, \
                         tc.tile_pool(name="tps", bufs=2, space="PSUM") as tps_p, \
                         tc.tile_pool(name="gps2", bufs=1, space="PSUM") as gps2_p:
                        wc2_sb = ph2.tile([P, kh, h], bf16)
                        nc.sync.dma_start(out=wc2_sb[:], in_=wc2[:].rearrange("p (k n) -> p k n", k=kh))
                        for b in range(nb):
                            gt = gat_p.tile([P, cpb, h], f8, tag="gt")
                            gather_half(gt, ag_out[1, "a"], idx1_sb, b, 0, ncA1[b])
                            gather_half(gt, ag_out[1, "b"], idx1_sb, b, ncA1[b], ncE1[b] - ncA1[b])
                            ws = wsl_p.tile([P, cpb, P], f8, tag="ws")
                            build_ws(ws, iota_w, dcol1_sb, b)
                            x2b = x2b_p.tile([P, kh, P], bf16, tag="x2b")
                            conv_block(gt, ws, ident, x2b, ncE1[b], cps_p, tps_p, agg_p)
                            ps2 = gps2_p.tile([P, h], f32, tag="g2ps")
                            for k in range(kh):
                                for hh in range(h // 512):
                                    nc.tensor.matmul(
                                        ps2[:, hh * 512:(hh + 1) * 512],
                                        lhsT=x2b[:, k, :],
                                        rhs=wc2_sb[:, k, hh * 512:(hh + 1) * 512],
                                        start=(k == 0),
                                        stop=(k == kh - 1),
                                    )
                            gsb = gout2_p.tile([P, h], f8, tag="gsb2")
                            nc.vector.tensor_scalar(
                                out=gsb[:], in0=ps2[:], scalar1=s_sb[:, nb + b:nb + b + 1],
                                scalar2=None, op0=mult,
                            )
                            dst_t = ag_in[2, "a"] if b < nba else ag_in[2, "b"]
                            roff = (b if b < nba else b - nba) * P
                            nc.sync.dma_start(out=dst_t[roff:roff + P, :], in_=gsb[:])

                    for hf in ("a", "b"):
                        nc.gpsimd.collective_compute(
                            "AllGather", mybir.AluOpType.bypass,
                            ins=[ag_in[2, hf][:]], outs=[ag_out[2, hf][:]], replica_groups=rg,
                        )

                    # ----- phase 3+4: conv2 per block -> x3[b]; out[b] = sigmoid(s3*(x3[b] @ W2))
                    if phases >= 3:
                        with tc.tile_pool(name="gat3", bufs=2) as gat3_p, \
                             tc.tile_pool(name="wsl3", bufs=2) as wsl3_p, \
                             tc.tile_pool(name="agg3", bufs=2) as agg3_p, \
                             tc.tile_pool(name="x3b", bufs=3) as x3b_p, \
                             tc.tile_pool(name="fout", bufs=3) as fout_p, \
                             tc.tile_pool(name="cps3", bufs=2, space="PSUM") as cps3_p, \
                             tc.tile_pool(name="tps3", bufs=2, space="PSUM") as tps3_p, \
                             tc.tile_pool(name="fps", bufs=2, space="PSUM") as fps_p:
                            for b in range(nb):
                                gt = gat3_p.tile([P, cpb, h], f8, tag="gt3")
                                gather_half(gt, ag_out[2, "a"], idx2_sb, b, 0, ncA2[b])
                                gather_half(gt, ag_out[2, "b"], idx2_sb, b, ncA2[b], ncE2[b] - ncA2[b])
                                ws = wsl3_p.tile([P, cpb, P], f8, tag="ws3")
                                build_ws(ws, iota_w, dcol2_sb, b)
                                x3b = x3b_p.tile([P, kh, P], f8, tag="x3b")
                                conv_block(gt, ws, ident, x3b, ncE2[b], cps3_p, tps3_p, agg3_p)
                                if phases >= 4:
                                    for gstart, gn in fgroups:
                                        o = fout_p.tile([P, 2048], bf16, tag="fo")
                                        for cs in range(0, gn, 512):
                                            cn = min(512, gn - cs)
                                            ps4 = fps_p.tile([P, 512], f32, tag="fps")
                                            for k2 in range(0, kh, 2):
                                                nc.tensor.matmul(
                                                    ps4[:, :cn],
                                                    lhsT=x3b[:, k2:k2 + 2, :],
                                                    rhs=w2_sb[:, k2:k2 + 2, gstart + cs:gstart + cs + cn],
                                                    start=(k2 == 0),
                                                    stop=(k2 == kh - 2),
                                                    perf_mode=DR,
                                                )
                                            nc.scalar.activation(
                                                out=o[:, cs:cs + cn], in_=ps4[:, :cn], func=Sigmoid,
                                                scale=s_sb[:, 2 * nb + b:2 * nb + b + 1],
                                            )
                                        nc.sync.dma_start(
                                            out=out_d[b * P:(b + 1) * P, gstart:gstart + gn],
                                            in_=o[:, :gn],
                                        )

    nc.compile()
    return nc


# ---------------------------------------------------------------- entry point

def _ensure_ntff_hook():
    """Register the axon NTFF profile hook if the image's antenv lacks it."""
    import contextlib
    import ctypes
    import sys
    import types

    try:
        from antenv.axon_hooks import get_axon_ntff_profile_hook  # noqa: F401
        return
    except ImportError:
        pass
    try:
        import antenv
    except ImportError:
        return
    mod = types.ModuleType("antenv.axon_hooks")
    holder = [None]
    mod.set_axon_ntff_profile_hook = lambda h: holder.__setitem__(0, h)
    mod.get_axon_ntff_profile_hook = lambda: holder[0]
    sys.modules["antenv.axon_hooks"] = mod
    antenv.axon_hooks = mod
    try:
        lib = ctypes.CDLL("/opt/axon/libaxon_pjrt.so")
    except OSError:
        return
    if not hasattr(lib, "axon_start_nrt_profile"):
        return
    lib.axon_start_nrt_profile.argtypes = [
        ctypes.POINTER(ctypes.c_int64),
        ctypes.c_size_t,
    ]
    lib.axon_start_nrt_profile.restype = ctypes.c_int64
    lib.axon_stop_nrt_profile.argtypes = [ctypes.c_char_p]
    lib.axon_stop_nrt_profile.restype = ctypes.c_int64

    @contextlib.contextmanager
    def _hook(output_dir, device_ids):
        import jax

        jax.devices()
        if device_ids:
            ids = (ctypes.c_int64 * len(device_ids))(*device_ids)
            rc = lib.axon_start_nrt_profile(ids, len(device_ids))
        else:
            rc = lib.axon_start_nrt_profile(None, 0)
        if rc != 0:
            raise RuntimeError(f"axon_start_nrt_profile rc={rc}")
        try:
            yield
        finally:
            n = lib.axon_stop_nrt_profile(str(output_dir).encode())
            print(f"profile: {n} file(s) written to {output_dir}", file=sys.stderr)

    holder[0] = _hook


def _run_hw(cfg, inputs, trace=False):
    if trace:
        _ensure_ntff_hook()
    cpb, metas, in_maps = prep_inputs(cfg, inputs)
    phases = int(os.environ.get("GNN_PHASES", "4"))
    nc = build_bass(cfg, cpb, metas, phases=phases)
    res = run_bass_kernel_spmd(nc, in_maps, core_ids=list(range(cfg.n_cores)), trace=trace)
    full = np.concatenate(
        [np.asarray(res.results[c]["out"]).astype(np.float32) for c in range(cfg.n_cores)],
        axis=0,
    )
    return full[: cfg.n_nodes], res


def kernel(**inputs) -> np.ndarray:
    trace = bool(int(os.environ.get("GNN_TRACE", "0")))
    out, res = _run_hw(FULL, inputs, trace=trace)
    if trace and res.exec_time_ns is not None:
        print(f"HW exec time: {res.exec_time_ns} ns")
    return out
